# revision 33
# baseline (speedup 1.0000x reference)
"""Trainium2 Bass kernel for nn_ActionNet (Wigner-D block-diag rotation + dense +
4x stride-2 conv_transpose decoder), data-parallel over 8 NeuronCores.

Math: real Wigner D^l(a,b,g) = Zr(a) @ dr(b) @ Zr(g), with
  Zr(t): Zr[m,m]=cos(mt), Zr[l+m,l-m]=sin(mt), Zr[l-m,l+m]=-sin(mt)
  dr(b)[u,v] = sum_q Cr_l[u,v,q] cos(b/2)^(2l-q) sin(b/2)^q,  Cr_l = Re(B C_l B^H)
conv_transpose(s=2,k=4,SAME) phases (verified vs jax):
  out[2p+d]: d=0 -> K[2] x[p] + K[0] x[p-1];  d=1 -> K[1] x[p] + K[3] x[p+1]
Trig: sx = sin(t/2 - pi/2) (safe LUT domain), cx = sqrt(1-sx^2);
  cos(t/2) = -sx, sin(t/2) = cx; then double-angle + recurrence for cos/sin(m t).
"""
import math
import sys
import types

import numpy as np

sys.path.insert(0, '/opt/trn_rl_repo')
import ml_dtypes

DEGREES = 6
NL = DEGREES + 1
R = 10
N_BATCH = 2048
NCORES = 8
NPC = N_BATCH // NCORES
NT = 32
PI = math.pi
TAPS = {0: [(2, 0), (0, -1)], 1: [(1, 0), (3, 1)]}
OFF49 = [l * l for l in range(NL + 1)]
OFF455 = np.cumsum([0] + [(2 * l + 1) ** 2 for l in range(NL)]).tolist()


def _install_axon_shim():
    if 'antenv.axon_hooks' in sys.modules:
        return
    mod = types.ModuleType('antenv.axon_hooks')
    _h = [None]
    mod.set_axon_ntff_profile_hook = lambda h: _h.__setitem__(0, h)
    mod.get_axon_ntff_profile_hook = lambda: _h[0]
    sys.modules['antenv.axon_hooks'] = mod
    try:
        import antenv
        antenv.axon_hooks = mod
        from trn_agent_boot.trn_boot import _ntff_profile_via_ctypes
        mod.set_axon_ntff_profile_hook(_ntff_profile_via_ctypes('/opt/axon/libaxon_pjrt.so'))
    except Exception:
        pass


def _wigner_coeffs(l):
    f = math.factorial
    n = 2 * l + 1
    C = np.zeros((n, n, n))
    for mp in range(-l, l + 1):
        for m in range(-l, l + 1):
            pref = math.sqrt(f(l + mp) * f(l - mp) * f(l + m) * f(l - m))
            for s in range(max(0, m - mp), min(l + m, l - mp) + 1):
                q = mp - m + 2 * s
                den = f(l + m - s) * f(s) * f(mp - m + s) * f(l - mp - s)
                C[mp + l, m + l, q] += ((-1.0) ** (mp - m + s)) * pref / den
    return C


def _real_basis(l):
    n = 2 * l + 1
    B = np.zeros((n, n), dtype=np.complex128)
    B[l, l] = 1.0
    isq = 1.0 / math.sqrt(2.0)
    for m in range(1, l + 1):
        B[l + m, l + m] = ((-1) ** m) * isq
        B[l + m, l - m] = isq
        B[l - m, l - m] = 1j * isq
        B[l - m, l + m] = -1j * ((-1) ** m) * isq
    return B


def _build_consts():
    CC = np.zeros((49, 456), np.float32)
    M14 = np.zeros((7, 980), np.float32)
    for l in range(NL):
        C = _wigner_coeffs(l)
        B = _real_basis(l)
        Cr = np.real(np.einsum('ua,abq,vb->uvq', B, C, B.conj())).astype(np.float32)
        n = 2 * l + 1
        CC[OFF49[l]:OFF49[l] + n, OFF455[l]:OFF455[l] + n * n] = \
            np.moveaxis(Cr, 2, 0).reshape(n, n * n)
        for ul in range(n):
            m = ul - l
            v = OFF49[l] + ul
            M14[abs(m), v * R:(v + 1) * R] = 1.0
            if m != 0:
                M14[abs(m), 490 + v * R:490 + (v + 1) * R] = float(np.sign(m))
    return CC, M14


def _flip49(x):
    out = np.empty_like(x)
    for l in range(NL):
        out[OFF49[l]:OFF49[l + 1]] = x[OFF49[l]:OFF49[l + 1]][::-1]
    return out


def _ki(d, s):
    for k, ss in TAPS[d]:
        if ss == s:
            return k
    return None


def _sis(d):
    return [s for s in (-1, 0, 1) if _ki(d, s) is not None]


def _prep_weights(item_rep, W, b, k1, b1, k2, b2, k3, b3, k4, b4):
    bf16 = ml_dtypes.bfloat16
    inp = {}
    inp['repM'] = np.concatenate(
        [np.tile(item_rep.reshape(-1), (7, 1)),
         np.tile(_flip49(item_rep).reshape(-1), (7, 1))], 1).astype(np.float32)
    CC, M14 = _build_consts()
    inp['CC'], inp['M14'] = CC, M14
    inp['ident'] = np.eye(128, dtype=np.float32)
    Wp = np.zeros((512, 4096), np.float32)
    Wp[:490] = W
    Wp[490] = b  # bias row; itemT row 490 is set to ones
    inp['Wb'] = np.ascontiguousarray(Wp.reshape(4, 128, 4096)).astype(bf16)
    k1s = np.zeros((128, 4, 4, 2, 128), np.float32)
    for pi4, (di, dj) in enumerate([(0, 0), (0, 1), (1, 0), (1, 1)]):
        tn = 0
        for si in _sis(di):
            for sj in _sis(dj):
                kk = k1[_ki(di, si), _ki(dj, sj)]
                for h in range(2):
                    k1s[:, pi4, tn, h, :] = kk[h * 128:(h + 1) * 128]
                tn += 1
    inp['k1s'] = k1s.astype(bf16)
    inp['b1'] = b1.reshape(128, 1).astype(np.float32)
    # conv2 col-tiled: lhsT [cin128, di, ri, t, (dj,ch,co32)]
    k2n = np.zeros((128, 2, 2, 2, 128), np.float32)
    for di in range(2):
        for ri in range(2):
            for tt in range(2):
                for dj in range(2):
                    kk = k2[_ki(di, di - 1 + ri), _ki(dj, dj - 1 + tt)]
                    k2n[:, di, ri, tt, dj * 64:(dj + 1) * 64] = kk
    inp['k2n'] = k2n.astype(bf16)
    inp['b2'] = np.tile(b2, 2).reshape(128, 1).astype(np.float32)
    # conv3 col-tiled: lhsT [(rsh,cin64)128, di, t, (jm=(2*qloc+dj),co32)]
    k3n = np.zeros((2, 64, 2, 2, 4, 32), np.float32)
    for di in range(2):
        for rsh in range(2):
            for tt in range(2):
                for qloc in range(2):
                    for dj in range(2):
                        k3n[rsh, :, di, tt, 2 * qloc + dj, :] = \
                            k3[_ki(di, di - 1 + rsh), _ki(dj, dj - 1 + tt)]
    inp['k3n'] = k3n.reshape(128, 2, 2, 128).astype(bf16)
    inp['b3'] = np.tile(b3, 4).reshape(128, 1).astype(np.float32)
    # conv4 col-tiled banded: tile t4=(di,jh) covers m=di*64+jh*32+2*(jout-16jh)+dj
    # K=(jm,c32) block window; 5 blocks per tile, 2 exact row steps
    k4n = np.zeros((4, 32, 4, 2, 5, 32), np.float32)
    for t4 in range(4):
        di, jh = t4 >> 1, t4 & 1
        for ri in range(2):
            ki = _ki(di, di - 1 + ri)
            for bb in range(5):
                for jm in range(4):
                    j = 4 * (bb + 3 * jh) + jm
                    for mloc in range(32):
                        jout = 16 * jh + (mloc >> 1)
                        dj = mloc & 1
                        sj = j - jout
                        if sj in (dj - 1, dj):
                            k4n[jm, :, t4, ri, bb, mloc] = k4[ki, _ki(dj, sj), :, 0]
    inp['k4n'] = k4n.reshape(128, 4, 2, 5, 32).astype(bf16)
    inp['b4'] = np.full((128, 1), float(b4[0]), np.float32)
    return inp


def _build():
    import concourse.bass as bass
    import concourse.mybir as mybir
    import concourse.tile as tile
    from concourse import bacc
    import contextlib

    dt = mybir.dt
    AF = mybir.ActivationFunctionType
    ALU = mybir.AluOpType
    f32, f32r, bf16 = dt.float32, dt.float32r, dt.bfloat16
    nc = bacc.Bacc("TRN2", target_bir_lowering=False, debug=False, num_devices=NCORES)

    def din(name, shape, dtype=f32):
        return nc.dram_tensor(name, list(shape), dtype, kind="ExternalInput").ap()

    ang = din('angles', [NPC, 3])
    repM_d = din('repM', [7, 980])
    CC_d = din('CC', [49, 456], f32r)
    M14_d = din('M14', [7, 980])
    id_d = din('ident', [128, 128])
    Wb_d = din('Wb', [4, 128, 4096], bf16)
    k1s_d = din('k1s', [128, 4, 4, 2, 128], bf16)
    b1_d = din('b1', [128, 1])
    k2n_d = din('k2n', [128, 2, 2, 2, 128], bf16)
    b2_d = din('b2', [128, 1])
    k3n_d = din('k3n', [128, 2, 2, 128], bf16)
    b3_d = din('b3', [128, 1])
    k4n_d = din('k4n', [128, 4, 2, 5, 32], bf16)
    b4_d = din('b4', [128, 1])
    out_d = nc.dram_tensor('out', [NPC, 64, 64], f32, kind="ExternalOutput").ap()

    def mk(t, off, dims):
        a = t[:]
        return bass.AP(tensor=a.tensor, offset=a.offset + off,
                       ap=[[a.ap[0][0], a.ap[0][1]]] + [[s, c] for s, c in dims])

    def mkp(t, p0, pn, off, dims):
        a = t[:]
        return bass.AP(tensor=a.tensor, offset=a.offset + p0 * a.ap[0][0] + off,
                       ap=[[a.ap[0][0], pn]] + [[s, c] for s, c in dims])

    with tile.TileContext(nc) as tc:
        ctx = contextlib.ExitStack()
        wp = ctx.enter_context(tc.tile_pool(name="wts", bufs=1))
        apl = ctx.enter_context(tc.tile_pool(name="acts", bufs=1))
        tp = ctx.enter_context(tc.tile_pool(name="tmp", bufs=1))
        pp = ctx.enter_context(tc.tile_pool(name="ps", bufs=8, space="PSUM"))

        def psum():
            return pp.tile([128, 512], f32, tag="ps", name="ps")

        def load(dram_ap, shape, dtype=f32, tag=None):
            t = wp.tile(shape, dtype, tag=tag)
            nc.sync.dma_start(out=t[:], in_=dram_ap)
            return t

        a3t = []
        for s in range(2):
            at = wp.tile([128, 3], f32, tag=f"a3_{s}", name=f"a3_{s}")
            nc.sync.dma_start(out=at[:], in_=ang[s * 128:(s + 1) * 128, :])
            a3t.append(at)
        # y-stage weights, split so matmul lhsT/rhs share base partition 0
        wdp = tc.tile_pool(name="wdense", bufs=1)
        wdpo = wdp.__enter__()
        def loadw(dram_ap, shape, dtype=f32, tag=None):
            t = wdpo.tile(shape, dtype, tag=tag, name=tag)
            nc.sync.dma_start(out=t[:], in_=dram_ap)
            return t
        repM = loadw(repM_d[:, :], [7, 980], tag="repM")
        CC = loadw(CC_d[:, :], [49, 456], f32r, tag="CC")
        M14 = loadw(M14_d[:, :], [7, 980], tag="M14")
        Fcs = wdpo.tile([7, 980], f32r, tag="Fcs", name="Fcs")
        nc.vector.tensor_mul(Fcs[:], M14[:], repM[:])
        ident = load(id_d[:, :], [128, 128], tag="ident")
        k1s = load(k1s_d[:, :, :, :, :], [128, 4, 4, 2, 128], bf16, tag="k1s")
        b1 = load(b1_d[:, :], [128, 1], tag="b1")
        k2n = load(k2n_d[:, :, :, :, :], [128, 2, 2, 2, 128], bf16, tag="k2n")
        b2 = load(b2_d[:, :], [128, 1], tag="b2")
        k3n = load(k3n_d[:, :, :, :], [128, 2, 2, 128], bf16, tag="k3n")
        b3 = load(b3_d[:, :], [128, 1], tag="b3")
        k4n = load(k4n_d[:, :, :, :, :], [128, 4, 2, 5, 32], bf16, tag="k4n")
        b4 = load(b4_d[:, :], [128, 1], tag="b4")

        c_half = wp.tile([128, 1], f32, tag="c_half", name="c_half")
        c_nhpi = wp.tile([128, 1], f32, tag="c_nhpi", name="c_nhpi")
        c_none = wp.tile([128, 1], f32, tag="c_none", name="c_none")
        c_one = wp.tile([128, 1], f32, tag="c_one", name="c_one")
        nc.vector.memset(c_half[:], 0.5)
        nc.vector.memset(c_nhpi[:], -PI / 2.0)
        nc.vector.memset(c_none[:], -1.0)
        nc.vector.memset(c_one[:], 1.0)
        c_zero = wp.tile([128, 1], f32, tag="c_zero", name="c_zero")
        nc.vector.memset(c_zero[:], 0.0)



        itemTb = [wdpo.tile([128, 256], bf16, tag=f"itemTb{kc}", name=f"itemTb{kc}") for kc in range(4)]
        nc.vector.memset(itemTb[3][:], 0.0)
        Wk = []
        for kc in range(4):
            wt = wdpo.tile([128, 4096], bf16, tag=f"Wk{kc}", name=f"Wk{kc}")
            nc.sync.dma_start(out=wt[:], in_=Wb_d[kc, :, :])
            Wk.append(wt)

        c1in = [apl.tile([128, NPC, 6, 6], bf16, tag=f"c1in{h}", name=f"c1in{h}") for h in range(2)]
        c1o = apl.tile([128, NT, 10, 10], bf16, tag="c1o", name="c1o")
        c2o2 = apl.tile([128, NT, 18, 2, 10], bf16, tag="c2o2", name="c2o2")
        c3o = apl.tile([128, NT, 34, 8], bf16, tag="c3o", name="c3o")
        otile = apl.tile([128, NT, 32], f32, tag="otile", name="otile")
        obuf = apl.tile([128, 8, 128], f32, tag="obuf", name="obuf")
        for h in range(2):
            nc.gpsimd.memset(c1in[h][:], 0.0)
        nc.gpsimd.memset(c1o[:].bitcast(f32), 0.0)
        nc.gpsimd.memset(c2o2[:].bitcast(f32), 0.0)
        nc.gpsimd.memset(c3o[:].bitcast(f32), 0.0)

        # ================= Wigner stage (one 128-sample batch) =================
        def wigner_a(s):
            a3 = a3t[s]
            # sx = sin(t/2 - pi/2), cx = sqrt(1 - sx^2)  for t = a, b, g
            sx = tp.tile([128, 3], f32, tag="sx", name="sx")
            cx = tp.tile([128, 3], f32, tag="cx", name="cx")
            sq = tp.tile([128, 3], f32, tag="sqt", name="sqt")
            nc.scalar.activation(sx[:], a3[:], AF.Sin, bias=c_nhpi[:], scale=c_half[:])
            nc.vector.tensor_mul(sq[:], sx[:], sx[:])
            nc.scalar.activation(cx[:], sq[:], AF.Sqrt, bias=c_one[:], scale=c_none[:])
            # half-angle of b: cb = -sx[:,1], sb = cx[:,1]
            cb = tp.tile([128, 1], f32, tag="cb", name="cb")
            sb = tp.tile([128, 1], f32, tag="sb", name="sb")
            nc.vector.tensor_scalar_mul(cb[:], sx[:, 1:2], -1.0)
            nc.vector.tensor_copy(sb[:], cx[:, 1:2])
            # full-angle cos/sin for a, g via double angle: cu = -sx, su = cx
            # c1 = 2 cu^2 - 1 ; s1 = 2 su cu
            cosT = tp.tile([128, 14], f32, tag=f"cosT{s}", name=f"cosT{s}")  # cols 0..6 cos(m a), 7..13 cos(m g)
            sinT = tp.tile([128, 14], f32, tag=f"sinT{s}", name=f"sinT{s}")
            nc.vector.memset(cosT[:, 0:1], 1.0)
            nc.vector.memset(cosT[:, 7:8], 1.0)
            nc.vector.memset(sinT[:, 0:1], 0.0)
            nc.vector.memset(sinT[:, 7:8], 0.0)
            cu = tp.tile([128, 2], f32, tag="cu", name="cu")
            su = tp.tile([128, 2], f32, tag="su", name="su")
            nc.vector.tensor_copy(cu[:], mk(sx, 0, [(2, 2)]))   # sx cols (a, g)
            nc.vector.tensor_copy(su[:], mk(cx, 0, [(2, 2)]))
            t0 = tp.tile([128, 2], f32, tag="t0", name="t0")
            nc.vector.tensor_mul(t0[:], cu[:], cu[:])
            c1 = tp.tile([128, 2], f32, tag="c1", name="c1")
            s1 = tp.tile([128, 2], f32, tag="s1", name="s1")
            nc.vector.tensor_scalar(c1[:], t0[:], 2.0, -1.0, op0=ALU.mult, op1=ALU.add)
            nc.vector.tensor_mul(t0[:], su[:], cu[:])
            nc.vector.tensor_scalar_mul(s1[:], t0[:], 2.0)
            # note cu = -sx is cos(t/2)? no: cu here = sx = -cos(t/2); but
            # c1 = 2 cu^2 - 1 = 2 cos^2(t/2) - 1 = cos(t) regardless of sign.
            # s1 = 2 su cu = 2 sin(t/2) (-cos(t/2)) = -sin(t) -> fix sign:
            nc.vector.tensor_scalar_mul(s1[:], s1[:], -1.0)
            cm = [None, (c1, 0)]
            # write m=1
            nc.vector.tensor_copy(mk(cosT, 1, [(7, 2)]), c1[:])
            nc.vector.tensor_copy(mk(sinT, 1, [(7, 2)]), s1[:])
            ta = tp.tile([128, 2], f32, tag="ta", name="ta")
            tb = tp.tile([128, 2], f32, tag="tb", name="tb")
            for m in range(2, 7):
                pcm = mk(cosT, m - 1, [(7, 2)])
                psm = mk(sinT, m - 1, [(7, 2)])
                nc.vector.tensor_mul(ta[:], pcm, c1[:])
                nc.vector.tensor_mul(tb[:], psm, s1[:])
                nc.vector.tensor_sub(mk(cosT, m, [(7, 2)]), ta[:], tb[:])
                nc.vector.tensor_mul(ta[:], psm, c1[:])
                nc.vector.tensor_mul(tb[:], pcm, s1[:])
                nc.vector.tensor_add(mk(sinT, m, [(7, 2)]), ta[:], tb[:])
            # power tables
            cpow = tp.tile([128, 13], f32, tag="cpow", name="cpow")
            spow = tp.tile([128, 13], f32, tag="spow", name="spow")
            for pw, base in ((cpow, cb), (spow, sb)):
                nc.vector.memset(pw[:, 0:1], 1.0)
                nc.vector.tensor_copy(pw[:, 1:2], base[:])
                xw = tp.tile([128, 1], f32, tag="xw", name="xw")
                nc.vector.tensor_mul(xw[:], base[:], base[:])
                nc.vector.tensor_scalar_mul(pw[:, 2:4], pw[:, 0:2], xw[:])
                nc.vector.tensor_mul(xw[:], xw[:], xw[:])
                nc.vector.tensor_scalar_mul(pw[:, 4:8], pw[:, 0:4], xw[:])
                nc.vector.tensor_mul(xw[:], xw[:], xw[:])
                nc.vector.tensor_scalar_mul(pw[:, 8:13], pw[:, 0:5], xw[:])
            # feat [128, 77] = [P49 | cosT 14 | sinT 14]
            feat = tp.tile([128, 103], f32, tag="feat", name="feat")
            for l in range(NL):
                n = 2 * l + 1
                nc.vector.tensor_mul(feat[:, OFF49[l]:OFF49[l] + n],
                                     mk(cpow, 2 * l, [(-1, n)]), spow[:, 0:n])
            nc.vector.tensor_copy(feat[:, 64:71], cosT[:, 7:14])
            nc.vector.tensor_copy(feat[:, 96:103], sinT[:, 7:14])
            # transpose -> base-0 lhsT tiles
            ptp = psum()
            nc.tensor.transpose(ptp[0:103, 0:128], feat[:, 0:103], ident[:])
            featP = tp.tile([49, 128], f32r, tag="featP", name="featP")
            featCG = tp.tile([7, 128], f32r, tag="featCG", name="featCG")
            featSG = tp.tile([7, 128], f32r, tag="featSG", name="featSG")
            nc.vector.tensor_copy(featP[:], ptp[0:49, 0:128])
            nc.vector.tensor_copy(featCG[:], ptp[64:71, 0:128])
            nc.vector.tensor_copy(featSG[:], ptp[96:103, 0:128])
            # dvec = P_all @ CC : [128, 455]
            pd = psum()
            nc.tensor.matmul(pd[:, 0:456], featP[:],
                             CC[:], start=True, stop=True)

            # y = T_g @ F : [128, 490]
            py = psum()
            nc.tensor.matmul(py[:, 0:490], featCG[:],
                             Fcs[:, 0:490], start=True, stop=False)
            nc.tensor.matmul(py[:, 0:490], featSG[:],
                             Fcs[:, 490:980], start=False, stop=True)
            return pd, py, cosT, sinT

        def wigner_b(s, pd, py, cosT, sinT):
            y = tp.tile([128, 490], f32, tag="y", name="y")
            nc.scalar.activation(y[:], py[:, 0:490], AF.Identity, bias=c_zero[:])
            dvec = tp.tile([128, 456], f32, tag="item", name="dvec")
            nc.scalar.activation(dvec[:], pd[:, 0:456], AF.Identity, bias=c_zero[:])
            # t2[n,(u,r)] = sum_v d[n,(u,v)] y[n,(v,r)] via wide mul + X-axis reduce
            t2 = tp.tile([128, 490], f32, tag="t2", name="t2")
            prod = tp.tile([128, 1690], f32, tag="prod", name="prod")
            nc.vector.tensor_scalar_mul(t2[:, 0:R], y[:, 0:R], dvec[:, 0:1])
            for l in range(1, NL):
                n = 2 * l + 1
                d_ap = mk(dvec, OFF455[l], [(n, n), (0, R), (1, n)])
                y_ap = mk(y, OFF49[l] * R, [(0, n), (1, R), (R, n)])
                nc.vector.tensor_mul(mk(prod, 0, [(n * R, n), (n, R), (1, n)]),
                                     d_ap, y_ap)
                nc.vector.tensor_reduce(
                    mk(t2, OFF49[l] * R, [(R, n), (1, R)]),
                    mk(prod, 0, [(n * R, n), (n, R), (1, n)]),
                    axis=mybir.AxisListType.X, op=ALU.add)
            # Za apply: item = caE*t2 + saE*flip(t2)
            nsin = tp.tile([128, 7], f32, tag="nsin", name="nsin")
            nc.scalar.activation(nsin[:], sinT[:, 0:7], AF.Identity,
                                 bias=c_zero[:], scale=c_none[:])
            caE = tp.tile([128, 490], f32, tag="caE", name="caE")
            saE = tp.tile([128, 490], f32, tag="saE", name="saE")
            for l in range(NL):
                n = 2 * l + 1
                base = OFF49[l] * R
                nc.scalar.activation(caE[:, base:base + (l + 1) * R],
                                     mk(cosT, l, [(-1, l + 1), (0, R)]),
                                     AF.Identity, bias=c_zero[:])
                nc.scalar.activation(saE[:, base:base + (l + 1) * R],
                                     mk(nsin, l, [(-1, l + 1), (0, R)]),
                                     AF.Identity, bias=c_zero[:])
                if l > 0:
                    nc.scalar.activation(caE[:, base + l * R:base + n * R],
                                         mk(cosT, 0, [(1, l + 1), (0, R)]),
                                         AF.Identity, bias=c_zero[:])
                    nc.scalar.activation(saE[:, base + l * R:base + n * R],
                                         mk(sinT, 0, [(1, l + 1), (0, R)]),
                                         AF.Identity, bias=c_zero[:])
            item = tp.tile([128, 496], f32, tag="item", name="item")
            tmp2 = tp.tile([128, 490], f32, tag="tmpf", name="tmpf")
            nc.vector.memset(item[:, 490:491], 1.0)  # ones col -> dense bias row
            nc.vector.tensor_mul(item[:, 0:490], caE[:], t2[:])
            for l in range(NL):
                n = 2 * l + 1
                base = OFF49[l] * R
                nc.vector.tensor_mul(tmp2[:, base:base + n * R], saE[:, base:base + n * R],
                                     mk(t2, base + (n - 1) * R, [(-R, n), (1, R)]))
            nc.vector.tensor_add(item[:, 0:490], item[:, 0:490], tmp2[:])
            for kc in range(4):
                cnt = 128 if kc < 3 else 107
                pit = psum()
                nc.tensor.transpose(pit[0:cnt, 0:128], item[:, kc * 128:kc * 128 + cnt],
                                    ident[:])
                nc.vector.tensor_copy(itemTb[kc][0:cnt, s * 128:(s + 1) * 128],
                                      pit[0:cnt, 0:128])

        # ================= dense -> c1in [(c-half),(n,6,6)] bf16 =================
        def dense_half(s):
            for yy in range(4):
                for h in range(2):
                    pd2 = psum()
                    for xx in range(4):
                        mc = yy * 8 + xx * 2 + h
                        for kc in range(4):
                            nc.tensor.matmul(pd2[:, xx * 128:(xx + 1) * 128],
                                             Wk[kc][:, mc * 128:(mc + 1) * 128],
                                             itemTb[kc][:, s * 128:(s + 1) * 128],
                                             start=(kc == 0), stop=(kc == 3))
                    srcap = bass.AP(tensor=pd2[:].tensor, offset=pd2[:].offset,
                                    ap=[list(pd2[:].ap[0]), [1, 128], [128, 4]])
                    nc.scalar.activation(mk(c1in[h], (1 + yy) * 6 + 1 + s * 128 * 36,
                                            [(36, 128), (1, 4)]),
                                         srcap, AF.Relu, bias=c_zero[:])

        # ================= conv stack, one n-tile of 32 =================
        def conv_tile(t):
            ns = t * NT
            # conv1: psum [c128, (n32,4,4)]
            for pi4, (di, dj) in enumerate([(0, 0), (0, 1), (1, 0), (1, 1)]):
                ps = psum()
                tn = 0
                for si in _sis(di):
                    for sj in _sis(dj):
                        for h in range(2):
                            rhs = mk(c1in[h], ns * 36 + (1 + si) * 6 + (1 + sj),
                                     [(36, NT), (6, 4), (1, 4)])
                            nc.tensor.matmul(ps[:, 0:512], k1s[:, pi4, tn, h, :], rhs,
                                             start=(tn == 0 and h == 0),
                                             stop=(tn == 3 and h == 1))
                        tn += 1
                dst1 = mk(c1o, (1 + di) * 10 + (1 + dj),
                          [(100, NT), (20, 4), (2, 4)])
                if pi4 % 2 == 0:
                    nc.scalar.activation(dst1, ps[:, 0:512], AF.Relu, bias=b1[:])
                else:
                    nc.vector.tensor_scalar(dst1, ps[:, 0:512], b1[:], 0.0,
                                            op0=ALU.add, op1=ALU.max)
            # conv2 col-tiled: psum [(dj,ch,c32) via 4 col tiles, (n8,p8,pj8)]
            # out rows 2p+di (psum per di); tile j4=(dj,ch); acc (ri,t) exact taps
            for g in range(4):
                for di in range(2):
                    ps = psum()
                    for st, (ri, tt) in enumerate(((0, 0), (1, 0), (0, 1), (1, 1))):
                        for dj in range(2):
                            rhs = mk(c1o, (g * 8) * 100 + (di + ri) * 10 + dj + tt,
                                     [(100, 8), (10, 8), (1, 8)])
                            nc.tensor.matmul(ps[64 * dj:64 * dj + 64, 0:512],
                                             k2n[:, di, ri, tt, 64 * dj:64 * dj + 64],
                                             rhs, start=(st == 0), stop=(st == 3),
                                             tile_position=(0, 64 * dj))
                    # evac rsh0: rows r=2p+di at slot r+1; (dj,pj)-split cols, pj+1
                    for dj in range(2):
                        dst = mkp(c2o2, 0, 64,
                                  (g * 8) * 360 + (di + 1) * 20 + dj * 10 + 1,
                                  [(360, 8), (40, 8), (1, 8)])
                        src = ps[64 * dj:64 * dj + 64, 0:512]
                        bslc = b2[64 * dj:64 * dj + 64, :]
                        if dj == 0:
                            nc.scalar.activation(dst, src, AF.Relu, bias=bslc)
                        else:
                            nc.vector.tensor_scalar(dst, src, bslc, 0.0,
                                                    op0=ALU.add, op1=ALU.max)
                # rsh1 half = rows shifted one slot down, via sbuf->sbuf DMA
                nc.sync.dma_start(
                    out=mkp(c2o2, 64, 64, (g * 8) * 360, [(360, 8), (1, 320)]),
                    in_=mkp(c2o2, 0, 64, (g * 8) * 360 + 20, [(360, 8), (1, 320)]))
            # conv3 col-tiled: psum [(jm4,c32) via 4 col tiles, (n4,p16,qp8)]
            # K=(rsh,c64) covers row taps; acc t covers col taps exactly
            for di in range(2):
                for g in range(8):
                    ps = psum()
                    for tt in range(2):
                        for jm in range(4):
                            qloc, dj = jm >> 1, jm & 1
                            o = qloc + dj - 1 + tt
                            rhs = mk(c2o2, (g * 4) * 360 + di * 20
                                     + (o & 1) * 10 + (o >> 1) + 1,
                                     [(360, 4), (20, 16), (1, 8)])
                            nc.tensor.matmul(ps[32 * jm:32 * jm + 32, 0:512],
                                             k3n[:, di, tt, 32 * jm:32 * jm + 32],
                                             rhs, start=(tt == 0), stop=(tt == 1),
                                             tile_position=(0, 32 * jm))
                    dst3 = mk(c3o, (g * 4) * 272 + (1 + di) * 8,
                              [(272, 4), (16, 16), (1, 8)])
                    if g % 2 == 0:
                        nc.vector.tensor_scalar(dst3, ps[:, 0:512], b3[:],
                                                0.0, op0=ALU.add, op1=ALU.max)
                    else:
                        nc.scalar.activation(dst3, ps[:, 0:512], AF.Relu, bias=b3[:])
            # conv4 col-tiled: tile t4=(di,jh); psum m = di*64+jh*32+2*joutloc+dj
            pss4 = [psum() for _ in range(2)]
            for st in range(10):
                bb, ri = st >> 1, st & 1
                for c2c in range(2):
                    for t4 in range(4):
                        di, jh = t4 >> 1, t4 & 1
                        si = di - 1 + ri
                        rhs = mk(c3o, (c2c * 16) * 272 + (1 + si) * 8 + bb + 3 * jh,
                                 [(272, 16), (8, 32)])
                        nc.tensor.matmul(pss4[c2c][32 * t4:32 * t4 + 32, 0:512],
                                         k4n[:, t4, ri, bb, :], rhs,
                                         start=(st == 0), stop=(st == 9),
                                         tile_position=(0, 32 * t4))
            for c2c in range(2):
                dst4 = mk(otile, (c2c * 16) * 32, [(32, 16), (1, 32)])
                if c2c == 0:
                    nc.scalar.activation(dst4, pss4[c2c][:, 0:512], AF.Identity,
                                         bias=b4[:])
                else:
                    nc.vector.tensor_scalar(dst4, pss4[c2c][:, 0:512], b4[:], 0.0,
                                            op0=ALU.add, op1=ALU.add)
            # transpose [m=(di,x), (n,io)] -> [(n4,io32), 128 contiguous pixels]
            # 4 transposes packed per psum bank, 1 wide evac per bank
            for cq in range(2):
                pst = psum()
                for cc in range(4):
                    cch = cq * 4 + cc
                    nc.tensor.transpose(pst[0:128, cc * 128:cc * 128 + 128],
                                        otile[:, cch * 4:cch * 4 + 4, :].bitcast(f32),
                                        ident[:])
                nc.scalar.activation(obuf[:, cq * 4:cq * 4 + 4, :], pst[:, 0:512],
                                     AF.Identity, bias=c_zero[:])
            dst = bass.AP(tensor=out_d.tensor, offset=out_d.offset + ns * 4096,
                          ap=[[4096, 4], [128, 32], [16384, 8], [1, 128]])
            nc.sync.dma_start(out=dst, in_=obuf[:])

        # ================= orchestration: overlap wigner(s=1) with convs =================
        ya0 = wigner_a(0)
        wigner_b(0, *ya0)
        dense_half(0)
        conv_tile(0)
        ya1 = wigner_a(1)
        conv_tile(1)
        wigner_b(1, *ya1)
        conv_tile(2)
        dense_half(1)
        conv_tile(3)
        for t in range(4, 8):
            conv_tile(t)
        wdp.__exit__(None, None, None)
        ctx.close()
    nc.compile()
    return nc


_NC_CACHE = {}


def kernel(angles, item_rep, W, b, k1, b1, k2, b2, k3, b3, k4, b4):
    _install_axon_shim()
    from concourse.bass_utils import run_bass_kernel_spmd
    if 'nc' not in _NC_CACHE:
        _NC_CACHE['nc'] = _build()
    nc = _NC_CACHE['nc']
    wts = _prep_weights(np.asarray(item_rep, np.float32), np.asarray(W, np.float32),
                        np.asarray(b, np.float32), np.asarray(k1, np.float32),
                        np.asarray(b1, np.float32), np.asarray(k2, np.float32),
                        np.asarray(b2, np.float32), np.asarray(k3, np.float32),
                        np.asarray(b3, np.float32), np.asarray(k4, np.float32),
                        np.asarray(b4, np.float32))
    angles = np.asarray(angles, np.float32)
    in_maps = []
    for c in range(NCORES):
        m = dict(wts)
        m['angles'] = np.ascontiguousarray(angles[c * NPC:(c + 1) * NPC])
        in_maps.append(m)
    res = run_bass_kernel_spmd(nc, in_maps, core_ids=list(range(NCORES)))
    return np.concatenate([r['out'][:, None, :, :] for r in res.results], axis=0)



# revision 36
# speedup vs baseline: 1.2488x; 1.2488x over previous
"""Trainium2 Bass kernel for nn_ActionNet (Wigner-D block-diag rotation + dense +
4x stride-2 conv_transpose decoder), data-parallel over 8 NeuronCores.

Math: real Wigner D^l(a,b,g) = Zr(a) @ dr(b) @ Zr(g), with
  Zr(t): Zr[m,m]=cos(mt), Zr[l+m,l-m]=sin(mt), Zr[l-m,l+m]=-sin(mt)
  dr(b)[u,v] = sum_q Cr_l[u,v,q] cos(b/2)^(2l-q) sin(b/2)^q,  Cr_l = Re(B C_l B^H)
conv_transpose(s=2,k=4,SAME) phases (verified vs jax):
  out[2p+d]: d=0 -> K[2] x[p] + K[0] x[p-1];  d=1 -> K[1] x[p] + K[3] x[p+1]
Trig: sx = sin(t/2 - pi/2) (safe LUT domain), cx = sqrt(1-sx^2);
  cos(t/2) = -sx, sin(t/2) = cx; then double-angle + recurrence for cos/sin(m t).
"""
import math
import sys
import types

import numpy as np

sys.path.insert(0, '/opt/trn_rl_repo')
import ml_dtypes

DEGREES = 6
NL = DEGREES + 1
R = 10
N_BATCH = 2048
NCORES = 8
NPC = N_BATCH // NCORES
NT = 32
PI = math.pi
TAPS = {0: [(2, 0), (0, -1)], 1: [(1, 0), (3, 1)]}
OFF49 = [l * l for l in range(NL + 1)]
OFF455 = np.cumsum([0] + [(2 * l + 1) ** 2 for l in range(NL)]).tolist()


def _install_axon_shim():
    if 'antenv.axon_hooks' in sys.modules:
        return
    mod = types.ModuleType('antenv.axon_hooks')
    _h = [None]
    mod.set_axon_ntff_profile_hook = lambda h: _h.__setitem__(0, h)
    mod.get_axon_ntff_profile_hook = lambda: _h[0]
    sys.modules['antenv.axon_hooks'] = mod
    try:
        import antenv
        antenv.axon_hooks = mod
        from trn_agent_boot.trn_boot import _ntff_profile_via_ctypes
        mod.set_axon_ntff_profile_hook(_ntff_profile_via_ctypes('/opt/axon/libaxon_pjrt.so'))
    except Exception:
        pass


def _wigner_coeffs(l):
    f = math.factorial
    n = 2 * l + 1
    C = np.zeros((n, n, n))
    for mp in range(-l, l + 1):
        for m in range(-l, l + 1):
            pref = math.sqrt(f(l + mp) * f(l - mp) * f(l + m) * f(l - m))
            for s in range(max(0, m - mp), min(l + m, l - mp) + 1):
                q = mp - m + 2 * s
                den = f(l + m - s) * f(s) * f(mp - m + s) * f(l - mp - s)
                C[mp + l, m + l, q] += ((-1.0) ** (mp - m + s)) * pref / den
    return C


def _real_basis(l):
    n = 2 * l + 1
    B = np.zeros((n, n), dtype=np.complex128)
    B[l, l] = 1.0
    isq = 1.0 / math.sqrt(2.0)
    for m in range(1, l + 1):
        B[l + m, l + m] = ((-1) ** m) * isq
        B[l + m, l - m] = isq
        B[l - m, l - m] = 1j * isq
        B[l - m, l + m] = -1j * ((-1) ** m) * isq
    return B


def _build_consts():
    CC = np.zeros((49, 456), np.float32)
    M14 = np.zeros((7, 980), np.float32)
    for l in range(NL):
        C = _wigner_coeffs(l)
        B = _real_basis(l)
        Cr = np.real(np.einsum('ua,abq,vb->uvq', B, C, B.conj())).astype(np.float32)
        n = 2 * l + 1
        CC[OFF49[l]:OFF49[l] + n, OFF455[l]:OFF455[l] + n * n] = \
            np.moveaxis(Cr, 2, 0).reshape(n, n * n)
        for ul in range(n):
            m = ul - l
            v = OFF49[l] + ul
            M14[abs(m), v * R:(v + 1) * R] = 1.0
            if m != 0:
                M14[abs(m), 490 + v * R:490 + (v + 1) * R] = float(np.sign(m))
    return CC, M14


def _flip49(x):
    out = np.empty_like(x)
    for l in range(NL):
        out[OFF49[l]:OFF49[l + 1]] = x[OFF49[l]:OFF49[l + 1]][::-1]
    return out


def _ki(d, s):
    for k, ss in TAPS[d]:
        if ss == s:
            return k
    return None


def _sis(d):
    return [s for s in (-1, 0, 1) if _ki(d, s) is not None]


def _prep_weights(item_rep, W, b, k1, b1, k2, b2, k3, b3, k4, b4):
    bf16 = ml_dtypes.bfloat16
    inp = {}
    inp['repM'] = np.concatenate(
        [np.tile(item_rep.reshape(-1), (7, 1)),
         np.tile(_flip49(item_rep).reshape(-1), (7, 1))], 1).astype(np.float32)
    CC, M14 = _build_consts()
    inp['CC'], inp['M14'] = CC, M14
    inp['ident'] = np.eye(128, dtype=np.float32)
    Wp = np.zeros((512, 4096), np.float32)
    Wp[:490] = W
    Wp[490] = b  # bias row; itemT row 490 is set to ones
    inp['Wb'] = np.ascontiguousarray(Wp.reshape(4, 128, 4096)).astype(bf16)
    k1s = np.zeros((128, 4, 4, 2, 128), np.float32)
    for pi4, (di, dj) in enumerate([(0, 0), (0, 1), (1, 0), (1, 1)]):
        tn = 0
        for si in _sis(di):
            for sj in _sis(dj):
                kk = k1[_ki(di, si), _ki(dj, sj)]
                for h in range(2):
                    k1s[:, pi4, tn, h, :] = kk[h * 128:(h + 1) * 128]
                tn += 1
    inp['k1s'] = k1s.astype(bf16)
    inp['b1'] = b1.reshape(128, 1).astype(np.float32)
    # conv2 col-tiled: lhsT [cin128, di, ri, t, (dj,ch,co32)]
    k2n = np.zeros((128, 2, 2, 2, 128), np.float32)
    for di in range(2):
        for ri in range(2):
            for tt in range(2):
                for dj in range(2):
                    kk = k2[_ki(di, di - 1 + ri), _ki(dj, dj - 1 + tt)]
                    k2n[:, di, ri, tt, dj * 64:(dj + 1) * 64] = kk
    inp['k2n'] = k2n.astype(bf16)
    inp['b2'] = np.tile(b2, 2).reshape(128, 1).astype(np.float32)
    # conv3 col-tiled: lhsT [(rsh,cin64)128, di, t, (jm=(2*qloc+dj),co32)]
    k3n = np.zeros((2, 64, 2, 2, 4, 32), np.float32)
    for di in range(2):
        for rsh in range(2):
            for tt in range(2):
                for qloc in range(2):
                    for dj in range(2):
                        k3n[rsh, :, di, tt, 2 * qloc + dj, :] = \
                            k3[_ki(di, di - 1 + rsh), _ki(dj, dj - 1 + tt)]
    inp['k3n'] = k3n.reshape(128, 2, 2, 128).astype(bf16)
    inp['b3'] = np.tile(b3, 4).reshape(128, 1).astype(np.float32)
    # conv4 col-tiled banded: tile t4=(di,jh) covers m=di*64+jh*32+2*(jout-16jh)+dj
    # K=(jm,c32) block window; 5 blocks per tile, 2 exact row steps
    k4n = np.zeros((4, 32, 4, 2, 5, 32), np.float32)
    for t4 in range(4):
        di, jh = t4 >> 1, t4 & 1
        for ri in range(2):
            ki = _ki(di, di - 1 + ri)
            for bb in range(5):
                for jm in range(4):
                    j = 4 * (bb + 3 * jh) + jm
                    for mloc in range(32):
                        jout = 16 * jh + (mloc >> 1)
                        dj = mloc & 1
                        sj = j - jout
                        if sj in (dj - 1, dj):
                            k4n[jm, :, t4, ri, bb, mloc] = k4[ki, _ki(dj, sj), :, 0]
    inp['k4n'] = k4n.reshape(128, 4, 2, 5, 32).astype(bf16)
    inp['b4'] = np.full((128, 1), float(b4[0]), np.float32)
    return inp


def _build():
    import concourse.bass as bass
    import concourse.mybir as mybir
    import concourse.tile as tile
    from concourse import bacc
    import contextlib

    dt = mybir.dt
    AF = mybir.ActivationFunctionType
    ALU = mybir.AluOpType
    f32, f32r, bf16 = dt.float32, dt.float32r, dt.bfloat16
    nc = bacc.Bacc("TRN2", target_bir_lowering=False, debug=False, num_devices=NCORES)

    def din(name, shape, dtype=f32):
        return nc.dram_tensor(name, list(shape), dtype, kind="ExternalInput").ap()

    ang = din('angles', [NPC, 3])
    repM_d = din('repM', [7, 980])
    CC_d = din('CC', [49, 456], f32r)
    M14_d = din('M14', [7, 980])
    id_d = din('ident', [128, 128])
    Wb_d = din('Wb', [4, 128, 4096], bf16)
    k1s_d = din('k1s', [128, 4, 4, 2, 128], bf16)
    b1_d = din('b1', [128, 1])
    k2n_d = din('k2n', [128, 2, 2, 2, 128], bf16)
    b2_d = din('b2', [128, 1])
    k3n_d = din('k3n', [128, 2, 2, 128], bf16)
    b3_d = din('b3', [128, 1])
    k4n_d = din('k4n', [128, 4, 2, 5, 32], bf16)
    b4_d = din('b4', [128, 1])
    out_d = nc.dram_tensor('out', [NPC, 64, 64], f32, kind="ExternalOutput").ap()

    def mk(t, off, dims):
        a = t[:]
        return bass.AP(tensor=a.tensor, offset=a.offset + off,
                       ap=[[a.ap[0][0], a.ap[0][1]]] + [[s, c] for s, c in dims])

    def mkp(t, p0, pn, off, dims):
        a = t[:]
        return bass.AP(tensor=a.tensor, offset=a.offset + p0 * a.ap[0][0] + off,
                       ap=[[a.ap[0][0], pn]] + [[s, c] for s, c in dims])

    with tile.TileContext(nc) as tc:
        ctx = contextlib.ExitStack()
        wp = ctx.enter_context(tc.tile_pool(name="wts", bufs=1))
        apl = ctx.enter_context(tc.tile_pool(name="acts", bufs=1))
        tp = ctx.enter_context(tc.tile_pool(name="tmp", bufs=1))
        pp = ctx.enter_context(tc.tile_pool(name="ps", bufs=8, space="PSUM"))

        def psum():
            return pp.tile([128, 512], f32, tag="ps", name="ps")

        def load(dram_ap, shape, dtype=f32, tag=None):
            t = wp.tile(shape, dtype, tag=tag)
            nc.sync.dma_start(out=t[:], in_=dram_ap)
            return t

        a3t = []
        for s in range(2):
            at = wp.tile([128, 3], f32, tag=f"a3_{s}", name=f"a3_{s}")
            nc.sync.dma_start(out=at[:], in_=ang[s * 128:(s + 1) * 128, :])
            a3t.append(at)
        # y-stage weights, split so matmul lhsT/rhs share base partition 0
        wdp = tc.tile_pool(name="wdense", bufs=1)
        wdpo = wdp.__enter__()
        def loadw(dram_ap, shape, dtype=f32, tag=None):
            t = wdpo.tile(shape, dtype, tag=tag, name=tag)
            nc.sync.dma_start(out=t[:], in_=dram_ap)
            return t
        repM = loadw(repM_d[:, :], [7, 980], tag="repM")
        CC = loadw(CC_d[:, :], [49, 456], f32r, tag="CC")
        M14 = loadw(M14_d[:, :], [7, 980], tag="M14")
        Fcs = wdpo.tile([7, 980], f32r, tag="Fcs", name="Fcs")
        nc.vector.tensor_mul(Fcs[:], M14[:], repM[:])
        ident = load(id_d[:, :], [128, 128], tag="ident")
        k1s = load(k1s_d[:, :, :, :, :], [128, 4, 4, 2, 128], bf16, tag="k1s")
        b1 = load(b1_d[:, :], [128, 1], tag="b1")
        k2n = load(k2n_d[:, :, :, :, :], [128, 2, 2, 2, 128], bf16, tag="k2n")
        b2 = load(b2_d[:, :], [128, 1], tag="b2")
        k3n = load(k3n_d[:, :, :, :], [128, 2, 2, 128], bf16, tag="k3n")
        b3 = load(b3_d[:, :], [128, 1], tag="b3")
        k4n = load(k4n_d[:, :, :, :, :], [128, 4, 2, 5, 32], bf16, tag="k4n")
        b4 = load(b4_d[:, :], [128, 1], tag="b4")

        c_half = wp.tile([128, 1], f32, tag="c_half", name="c_half")
        c_nhpi = wp.tile([128, 1], f32, tag="c_nhpi", name="c_nhpi")
        c_none = wp.tile([128, 1], f32, tag="c_none", name="c_none")
        c_one = wp.tile([128, 1], f32, tag="c_one", name="c_one")
        nc.vector.memset(c_half[:], 0.5)
        nc.vector.memset(c_nhpi[:], -PI / 2.0)
        nc.vector.memset(c_none[:], -1.0)
        nc.vector.memset(c_one[:], 1.0)
        c_zero = wp.tile([128, 1], f32, tag="c_zero", name="c_zero")
        nc.vector.memset(c_zero[:], 0.0)



        itemTb = [wdpo.tile([128, 256], bf16, tag=f"itemTb{kc}", name=f"itemTb{kc}") for kc in range(4)]
        nc.vector.memset(itemTb[3][:], 0.0)
        Wk = []
        for kc in range(4):
            wt = wdpo.tile([128, 4096], bf16, tag=f"Wk{kc}", name=f"Wk{kc}")
            nc.sync.dma_start(out=wt[:], in_=Wb_d[kc, :, :])
            Wk.append(wt)

        c1in = [apl.tile([128, NPC, 6, 6], bf16, tag=f"c1in{h}", name=f"c1in{h}") for h in range(2)]
        c1o = apl.tile([128, NT, 10, 10], bf16, tag="c1o", name="c1o")
        c2o2 = apl.tile([128, NT, 18, 2, 10], bf16, tag="c2o2", name="c2o2")
        c3o = apl.tile([128, NT, 8, 34], bf16, tag="c3o", name="c3o")
        otile = apl.tile([128, NT, 32], f32, tag="otile", name="otile")
        obuf = apl.tile([128, 8, 128], f32, tag="obuf", name="obuf")
        for h in range(2):
            nc.gpsimd.memset(c1in[h][:], 0.0)
        nc.gpsimd.memset(c1o[:].bitcast(f32), 0.0)
        nc.gpsimd.memset(c2o2[:].bitcast(f32), 0.0)
        nc.gpsimd.memset(c3o[:].bitcast(f32), 0.0)

        # ================= Wigner stage (one 128-sample batch) =================
        def wigner_a(s):
            a3 = a3t[s]
            # sx = sin(t/2 - pi/2), cx = sqrt(1 - sx^2)  for t = a, b, g
            sx = tp.tile([128, 3], f32, tag="sx", name="sx")
            cx = tp.tile([128, 3], f32, tag="cx", name="cx")
            sq = tp.tile([128, 3], f32, tag="sqt", name="sqt")
            nc.scalar.activation(sx[:], a3[:], AF.Sin, bias=c_nhpi[:], scale=c_half[:])
            nc.vector.tensor_mul(sq[:], sx[:], sx[:])
            nc.scalar.activation(cx[:], sq[:], AF.Sqrt, bias=c_one[:], scale=c_none[:])
            # half-angle of b: cb = -sx[:,1], sb = cx[:,1]
            cb = tp.tile([128, 1], f32, tag="cb", name="cb")
            sb = tp.tile([128, 1], f32, tag="sb", name="sb")
            nc.vector.tensor_scalar_mul(cb[:], sx[:, 1:2], -1.0)
            nc.vector.tensor_copy(sb[:], cx[:, 1:2])
            # full-angle cos/sin for a, g via double angle: cu = -sx, su = cx
            # c1 = 2 cu^2 - 1 ; s1 = 2 su cu
            cosT = tp.tile([128, 14], f32, tag=f"cosT{s}", name=f"cosT{s}")  # cols 0..6 cos(m a), 7..13 cos(m g)
            sinT = tp.tile([128, 14], f32, tag=f"sinT{s}", name=f"sinT{s}")
            nc.vector.memset(cosT[:, 0:1], 1.0)
            nc.vector.memset(cosT[:, 7:8], 1.0)
            nc.vector.memset(sinT[:, 0:1], 0.0)
            nc.vector.memset(sinT[:, 7:8], 0.0)
            cu = tp.tile([128, 2], f32, tag="cu", name="cu")
            su = tp.tile([128, 2], f32, tag="su", name="su")
            nc.vector.tensor_copy(cu[:], mk(sx, 0, [(2, 2)]))   # sx cols (a, g)
            nc.vector.tensor_copy(su[:], mk(cx, 0, [(2, 2)]))
            t0 = tp.tile([128, 2], f32, tag="t0", name="t0")
            nc.vector.tensor_mul(t0[:], cu[:], cu[:])
            c1 = tp.tile([128, 2], f32, tag="c1", name="c1")
            s1 = tp.tile([128, 2], f32, tag="s1", name="s1")
            nc.vector.tensor_scalar(c1[:], t0[:], 2.0, -1.0, op0=ALU.mult, op1=ALU.add)
            nc.vector.tensor_mul(t0[:], su[:], cu[:])
            nc.vector.tensor_scalar_mul(s1[:], t0[:], 2.0)
            # note cu = -sx is cos(t/2)? no: cu here = sx = -cos(t/2); but
            # c1 = 2 cu^2 - 1 = 2 cos^2(t/2) - 1 = cos(t) regardless of sign.
            # s1 = 2 su cu = 2 sin(t/2) (-cos(t/2)) = -sin(t) -> fix sign:
            nc.vector.tensor_scalar_mul(s1[:], s1[:], -1.0)
            cm = [None, (c1, 0)]
            # write m=1
            nc.vector.tensor_copy(mk(cosT, 1, [(7, 2)]), c1[:])
            nc.vector.tensor_copy(mk(sinT, 1, [(7, 2)]), s1[:])
            ta = tp.tile([128, 2], f32, tag="ta", name="ta")
            tb = tp.tile([128, 2], f32, tag="tb", name="tb")
            for m in range(2, 7):
                pcm = mk(cosT, m - 1, [(7, 2)])
                psm = mk(sinT, m - 1, [(7, 2)])
                nc.vector.tensor_mul(ta[:], pcm, c1[:])
                nc.vector.tensor_mul(tb[:], psm, s1[:])
                nc.vector.tensor_sub(mk(cosT, m, [(7, 2)]), ta[:], tb[:])
                nc.vector.tensor_mul(ta[:], psm, c1[:])
                nc.vector.tensor_mul(tb[:], pcm, s1[:])
                nc.vector.tensor_add(mk(sinT, m, [(7, 2)]), ta[:], tb[:])
            # power tables
            cpow = tp.tile([128, 13], f32, tag="cpow", name="cpow")
            spow = tp.tile([128, 13], f32, tag="spow", name="spow")
            for pw, base in ((cpow, cb), (spow, sb)):
                nc.vector.memset(pw[:, 0:1], 1.0)
                nc.vector.tensor_copy(pw[:, 1:2], base[:])
                xw = tp.tile([128, 1], f32, tag="xw", name="xw")
                nc.vector.tensor_mul(xw[:], base[:], base[:])
                nc.vector.tensor_scalar_mul(pw[:, 2:4], pw[:, 0:2], xw[:])
                nc.vector.tensor_mul(xw[:], xw[:], xw[:])
                nc.vector.tensor_scalar_mul(pw[:, 4:8], pw[:, 0:4], xw[:])
                nc.vector.tensor_mul(xw[:], xw[:], xw[:])
                nc.vector.tensor_scalar_mul(pw[:, 8:13], pw[:, 0:5], xw[:])
            # feat [128, 77] = [P49 | cosT 14 | sinT 14]
            feat = tp.tile([128, 103], f32, tag="feat", name="feat")
            for l in range(NL):
                n = 2 * l + 1
                nc.vector.tensor_mul(feat[:, OFF49[l]:OFF49[l] + n],
                                     mk(cpow, 2 * l, [(-1, n)]), spow[:, 0:n])
            nc.vector.tensor_copy(feat[:, 64:71], cosT[:, 7:14])
            nc.vector.tensor_copy(feat[:, 96:103], sinT[:, 7:14])
            # transpose -> base-0 lhsT tiles
            ptp = psum()
            nc.tensor.transpose(ptp[0:103, 0:128], feat[:, 0:103], ident[:])
            featP = tp.tile([49, 128], f32r, tag="featP", name="featP")
            featCG = tp.tile([7, 128], f32r, tag="featCG", name="featCG")
            featSG = tp.tile([7, 128], f32r, tag="featSG", name="featSG")
            nc.vector.tensor_copy(featP[:], ptp[0:49, 0:128])
            nc.vector.tensor_copy(featCG[:], ptp[64:71, 0:128])
            nc.vector.tensor_copy(featSG[:], ptp[96:103, 0:128])
            # dvec = P_all @ CC : [128, 455]
            pd = psum()
            nc.tensor.matmul(pd[:, 0:456], featP[:],
                             CC[:], start=True, stop=True)

            # y = T_g @ F : [128, 490]
            py = psum()
            nc.tensor.matmul(py[:, 0:490], featCG[:],
                             Fcs[:, 0:490], start=True, stop=False)
            nc.tensor.matmul(py[:, 0:490], featSG[:],
                             Fcs[:, 490:980], start=False, stop=True)
            # evacuate psums here so none are held across interleaved conv tiles
            y = tp.tile([128, 490], bf16, tag="y", name="y")
            nc.scalar.activation(y[:], py[:, 0:490], AF.Identity, bias=c_zero[:])
            dvec = tp.tile([128, 456], bf16, tag="dv", name="dvec")
            nc.scalar.activation(dvec[:], pd[:, 0:456], AF.Identity, bias=c_zero[:])
            return y, dvec, cosT, sinT

        def wigner_b(s, y, dvec, cosT, sinT):
            # t2[n,(u,r)] = sum_v d[n,(u,v)] y[n,(v,r)] via wide mul + X-axis reduce
            t2 = tp.tile([128, 490], f32, tag="t2", name="t2")
            prod = tp.tile([128, 1690], bf16, tag="prod", name="prod")
            nc.vector.tensor_mul(t2[:, 0:R], y[:, 0:R], mk(dvec, 0, [(0, R)]))
            for l in range(1, NL):
                n = 2 * l + 1
                d_ap = mk(dvec, OFF455[l], [(n, n), (0, R), (1, n)])
                y_ap = mk(y, OFF49[l] * R, [(0, n), (1, R), (R, n)])
                nc.vector.tensor_mul(mk(prod, 0, [(n * R, n), (n, R), (1, n)]),
                                     d_ap, y_ap)
                nc.vector.tensor_reduce(
                    mk(t2, OFF49[l] * R, [(R, n), (1, R)]),
                    mk(prod, 0, [(n * R, n), (n, R), (1, n)]),
                    axis=mybir.AxisListType.X, op=ALU.add)
            # Za apply: item = caE*t2 + saE*flip(t2)
            nsin = tp.tile([128, 7], f32, tag="nsin", name="nsin")
            nc.scalar.activation(nsin[:], sinT[:, 0:7], AF.Identity,
                                 bias=c_zero[:], scale=c_none[:])
            caE = tp.tile([128, 490], f32, tag="caE", name="caE")
            saE = tp.tile([128, 490], f32, tag="saE", name="saE")
            for l in range(NL):
                n = 2 * l + 1
                base = OFF49[l] * R
                nc.scalar.activation(caE[:, base:base + (l + 1) * R],
                                     mk(cosT, l, [(-1, l + 1), (0, R)]),
                                     AF.Identity, bias=c_zero[:])
                nc.scalar.activation(saE[:, base:base + (l + 1) * R],
                                     mk(nsin, l, [(-1, l + 1), (0, R)]),
                                     AF.Identity, bias=c_zero[:])
                if l > 0:
                    nc.scalar.activation(caE[:, base + l * R:base + n * R],
                                         mk(cosT, 0, [(1, l + 1), (0, R)]),
                                         AF.Identity, bias=c_zero[:])
                    nc.scalar.activation(saE[:, base + l * R:base + n * R],
                                         mk(sinT, 0, [(1, l + 1), (0, R)]),
                                         AF.Identity, bias=c_zero[:])
            item = tp.tile([128, 496], f32, tag="item", name="item")
            tmp2 = tp.tile([128, 490], f32, tag="tmpf", name="tmpf")
            nc.vector.memset(item[:, 490:491], 1.0)  # ones col -> dense bias row
            nc.vector.tensor_mul(item[:, 0:490], caE[:], t2[:])
            for l in range(NL):
                n = 2 * l + 1
                base = OFF49[l] * R
                nc.vector.tensor_mul(tmp2[:, base:base + n * R], saE[:, base:base + n * R],
                                     mk(t2, base + (n - 1) * R, [(-R, n), (1, R)]))
            nc.vector.tensor_add(item[:, 0:490], item[:, 0:490], tmp2[:])
            for kc in range(4):
                cnt = 128 if kc < 3 else 107
                pit = psum()
                nc.tensor.transpose(pit[0:cnt, 0:128], item[:, kc * 128:kc * 128 + cnt],
                                    ident[:])
                nc.vector.tensor_copy(itemTb[kc][0:cnt, s * 128:(s + 1) * 128],
                                      pit[0:cnt, 0:128])

        # ================= dense -> c1in [(c-half),(n,6,6)] bf16 =================
        def dense_half(s):
            for yy in range(4):
                for h in range(2):
                    pd2 = psum()
                    for xx in range(4):
                        mc = yy * 8 + xx * 2 + h
                        for kc in range(4):
                            nc.tensor.matmul(pd2[:, xx * 128:(xx + 1) * 128],
                                             Wk[kc][:, mc * 128:(mc + 1) * 128],
                                             itemTb[kc][:, s * 128:(s + 1) * 128],
                                             start=(kc == 0), stop=(kc == 3))
                    srcap = bass.AP(tensor=pd2[:].tensor, offset=pd2[:].offset,
                                    ap=[list(pd2[:].ap[0]), [1, 128], [128, 4]])
                    nc.scalar.activation(mk(c1in[h], (1 + yy) * 6 + 1 + s * 128 * 36,
                                            [(36, 128), (1, 4)]),
                                         srcap, AF.Relu, bias=c_zero[:])

        # ================= conv stack, one n-tile of 32 =================
        def conv_tile(t):
            ns = t * NT
            # conv1: psum [c128, (n32,4,4)]
            for pi4, (di, dj) in enumerate([(0, 0), (0, 1), (1, 0), (1, 1)]):
                ps = psum()
                tn = 0
                for si in _sis(di):
                    for sj in _sis(dj):
                        for h in range(2):
                            rhs = mk(c1in[h], ns * 36 + (1 + si) * 6 + (1 + sj),
                                     [(36, NT), (6, 4), (1, 4)])
                            nc.tensor.matmul(ps[:, 0:512], k1s[:, pi4, tn, h, :], rhs,
                                             start=(tn == 0 and h == 0),
                                             stop=(tn == 3 and h == 1))
                        tn += 1
                dst1 = mk(c1o, (1 + di) * 10 + (1 + dj),
                          [(100, NT), (20, 4), (2, 4)])
                if pi4 % 2 == 0:
                    nc.scalar.activation(dst1, ps[:, 0:512], AF.Relu, bias=b1[:])
                else:
                    nc.vector.tensor_scalar(dst1, ps[:, 0:512], b1[:], 0.0,
                                            op0=ALU.add, op1=ALU.max)
            # conv2 col-tiled: psum [(dj,ch,c32) via 4 col tiles, (n8,p8,pj8)]
            # out rows 2p+di (psum per di); tile j4=(dj,ch); acc (ri,t) exact taps
            for g in range(4):
                for di in range(2):
                    ps = psum()
                    for st, (ri, tt) in enumerate(((0, 0), (1, 0), (0, 1), (1, 1))):
                        for dj in range(2):
                            rhs = mk(c1o, (g * 8) * 100 + (di + ri) * 10 + dj + tt,
                                     [(100, 8), (10, 8), (1, 8)])
                            nc.tensor.matmul(ps[64 * dj:64 * dj + 64, 0:512],
                                             k2n[:, di, ri, tt, 64 * dj:64 * dj + 64],
                                             rhs, start=(st == 0), stop=(st == 3),
                                             tile_position=(0, 64 * dj))
                    # evac rsh0: rows r=2p+di at slot r+1; (dj,pj)-split cols, pj+1
                    for dj in range(2):
                        dst = mkp(c2o2, 0, 64,
                                  (g * 8) * 360 + (di + 1) * 20 + dj * 10 + 1,
                                  [(360, 8), (40, 8), (1, 8)])
                        src = ps[64 * dj:64 * dj + 64, 0:512]
                        bslc = b2[64 * dj:64 * dj + 64, :]
                        if dj == 0:
                            nc.scalar.activation(dst, src, AF.Relu, bias=bslc)
                        else:
                            nc.vector.tensor_scalar(dst, src, bslc, 0.0,
                                                    op0=ALU.add, op1=ALU.max)
                # rsh1 half = rows shifted one slot down, via sbuf->sbuf DMA
                nc.sync.dma_start(
                    out=mkp(c2o2, 64, 64, (g * 8) * 360, [(360, 8), (1, 320)]),
                    in_=mkp(c2o2, 0, 64, (g * 8) * 360 + 20, [(360, 8), (1, 320)]))
            # conv3 col-tiled: psum [(jm4,c32) via 4 col tiles, (n4,p16,qp8)]
            # K=(rsh,c64) covers row taps; acc t covers col taps exactly
            for di in range(2):
                for g in range(8):
                    ps = psum()
                    for tt in range(2):
                        for jm in range(4):
                            qloc, dj = jm >> 1, jm & 1
                            o = qloc + dj - 1 + tt
                            rhs = mk(c2o2, (g * 4) * 360 + di * 20
                                     + (o & 1) * 10 + (o >> 1) + 1,
                                     [(360, 4), (20, 16), (1, 8)])
                            nc.tensor.matmul(ps[32 * jm:32 * jm + 32, 0:512],
                                             k3n[:, di, tt, 32 * jm:32 * jm + 32],
                                             rhs, start=(tt == 0), stop=(tt == 1),
                                             tile_position=(0, 32 * jm))
                    dst3 = mk(c3o, (g * 4) * 272 + 1 + di,
                              [(272, 4), (34, 8), (2, 16)])
                    ps3 = bass.AP(tensor=ps[:].tensor, offset=ps[:].offset,
                                  ap=[list(ps[:].ap[0]), [128, 4], [1, 8], [8, 16]])
                    if g % 2 == 0:
                        nc.vector.tensor_scalar(dst3, ps3, b3[:],
                                                0.0, op0=ALU.add, op1=ALU.max)
                    else:
                        nc.scalar.activation(dst3, ps3, AF.Relu, bias=b3[:])
            # conv4 col-tiled: tile t4=(di,jh); psum m = di*64+jh*32+2*joutloc+dj
            pss4 = [psum() for _ in range(2)]
            for st in range(10):
                bb, ri = st >> 1, st & 1
                for c2c in range(2):
                    for t4 in range(4):
                        di, jh = t4 >> 1, t4 & 1
                        si = di - 1 + ri
                        rhs = mk(c3o, (c2c * 16) * 272 + (bb + 3 * jh) * 34 + 1 + si,
                                 [(272, 16), (1, 32)])
                        nc.tensor.matmul(pss4[c2c][32 * t4:32 * t4 + 32, 0:512],
                                         k4n[:, t4, ri, bb, :], rhs,
                                         start=(st == 0), stop=(st == 9),
                                         tile_position=(0, 32 * t4))
            for c2c in range(2):
                dst4 = mk(otile, (c2c * 16) * 32, [(32, 16), (1, 32)])
                if c2c == 0:
                    nc.scalar.activation(dst4, pss4[c2c][:, 0:512], AF.Identity,
                                         bias=b4[:])
                else:
                    nc.vector.tensor_scalar(dst4, pss4[c2c][:, 0:512], b4[:], 0.0,
                                            op0=ALU.add, op1=ALU.add)
            # transpose [m=(di,x), (n,io)] -> [(n4,io32), 128 contiguous pixels]
            # 4 transposes packed per psum bank, 1 wide evac per bank
            for cq in range(2):
                pst = psum()
                for cc in range(4):
                    cch = cq * 4 + cc
                    nc.tensor.transpose(pst[0:128, cc * 128:cc * 128 + 128],
                                        otile[:, cch * 4:cch * 4 + 4, :].bitcast(f32),
                                        ident[:])
                nc.scalar.activation(obuf[:, cq * 4:cq * 4 + 4, :], pst[:, 0:512],
                                     AF.Identity, bias=c_zero[:])
            dst = bass.AP(tensor=out_d.tensor, offset=out_d.offset + ns * 4096,
                          ap=[[4096, 4], [128, 32], [16384, 8], [1, 128]])
            nc.sync.dma_start(out=dst, in_=obuf[:])

        # ================= orchestration: overlap wigner(s=1) with convs =================
        ya0 = wigner_a(0)
        wigner_b(0, *ya0)
        dense_half(0)
        conv_tile(0)
        ya1 = wigner_a(1)
        conv_tile(1)
        wigner_b(1, *ya1)
        conv_tile(2)
        dense_half(1)
        conv_tile(3)
        for t in range(4, 8):
            conv_tile(t)
        wdp.__exit__(None, None, None)
        ctx.close()
    nc.compile()
    return nc


_NC_CACHE = {}


def kernel(angles, item_rep, W, b, k1, b1, k2, b2, k3, b3, k4, b4):
    _install_axon_shim()
    from concourse.bass_utils import run_bass_kernel_spmd
    if 'nc' not in _NC_CACHE:
        _NC_CACHE['nc'] = _build()
    nc = _NC_CACHE['nc']
    wts = _prep_weights(np.asarray(item_rep, np.float32), np.asarray(W, np.float32),
                        np.asarray(b, np.float32), np.asarray(k1, np.float32),
                        np.asarray(b1, np.float32), np.asarray(k2, np.float32),
                        np.asarray(b2, np.float32), np.asarray(k3, np.float32),
                        np.asarray(b3, np.float32), np.asarray(k4, np.float32),
                        np.asarray(b4, np.float32))
    angles = np.asarray(angles, np.float32)
    in_maps = []
    for c in range(NCORES):
        m = dict(wts)
        m['angles'] = np.ascontiguousarray(angles[c * NPC:(c + 1) * NPC])
        in_maps.append(m)
    res = run_bass_kernel_spmd(nc, in_maps, core_ids=list(range(NCORES)))
    return np.concatenate([r['out'][:, None, :, :] for r in res.results], axis=0)



# revision 38
# speedup vs baseline: 1.2528x; 1.0032x over previous
"""Trainium2 Bass kernel for nn_ActionNet (Wigner-D block-diag rotation + dense +
4x stride-2 conv_transpose decoder), data-parallel over 8 NeuronCores.

Math: real Wigner D^l(a,b,g) = Zr(a) @ dr(b) @ Zr(g), with
  Zr(t): Zr[m,m]=cos(mt), Zr[l+m,l-m]=sin(mt), Zr[l-m,l+m]=-sin(mt)
  dr(b)[u,v] = sum_q Cr_l[u,v,q] cos(b/2)^(2l-q) sin(b/2)^q,  Cr_l = Re(B C_l B^H)
conv_transpose(s=2,k=4,SAME) phases (verified vs jax):
  out[2p+d]: d=0 -> K[2] x[p] + K[0] x[p-1];  d=1 -> K[1] x[p] + K[3] x[p+1]
Trig: sx = sin(t/2 - pi/2) (safe LUT domain), cx = sqrt(1-sx^2);
  cos(t/2) = -sx, sin(t/2) = cx; then double-angle + recurrence for cos/sin(m t).
"""
import math
import sys
import types

import numpy as np

sys.path.insert(0, '/opt/trn_rl_repo')
import ml_dtypes

DEGREES = 6
NL = DEGREES + 1
R = 10
N_BATCH = 2048
NCORES = 8
NPC = N_BATCH // NCORES
NT = 32
PI = math.pi
TAPS = {0: [(2, 0), (0, -1)], 1: [(1, 0), (3, 1)]}
OFF49 = [l * l for l in range(NL + 1)]
OFF455 = np.cumsum([0] + [(2 * l + 1) ** 2 for l in range(NL)]).tolist()


def _install_axon_shim():
    if 'antenv.axon_hooks' in sys.modules:
        return
    mod = types.ModuleType('antenv.axon_hooks')
    _h = [None]
    mod.set_axon_ntff_profile_hook = lambda h: _h.__setitem__(0, h)
    mod.get_axon_ntff_profile_hook = lambda: _h[0]
    sys.modules['antenv.axon_hooks'] = mod
    try:
        import antenv
        antenv.axon_hooks = mod
        from trn_agent_boot.trn_boot import _ntff_profile_via_ctypes
        mod.set_axon_ntff_profile_hook(_ntff_profile_via_ctypes('/opt/axon/libaxon_pjrt.so'))
    except Exception:
        pass


def _wigner_coeffs(l):
    f = math.factorial
    n = 2 * l + 1
    C = np.zeros((n, n, n))
    for mp in range(-l, l + 1):
        for m in range(-l, l + 1):
            pref = math.sqrt(f(l + mp) * f(l - mp) * f(l + m) * f(l - m))
            for s in range(max(0, m - mp), min(l + m, l - mp) + 1):
                q = mp - m + 2 * s
                den = f(l + m - s) * f(s) * f(mp - m + s) * f(l - mp - s)
                C[mp + l, m + l, q] += ((-1.0) ** (mp - m + s)) * pref / den
    return C


def _real_basis(l):
    n = 2 * l + 1
    B = np.zeros((n, n), dtype=np.complex128)
    B[l, l] = 1.0
    isq = 1.0 / math.sqrt(2.0)
    for m in range(1, l + 1):
        B[l + m, l + m] = ((-1) ** m) * isq
        B[l + m, l - m] = isq
        B[l - m, l - m] = 1j * isq
        B[l - m, l + m] = -1j * ((-1) ** m) * isq
    return B


def _build_consts():
    CC = np.zeros((49, 456), np.float32)
    M14 = np.zeros((7, 980), np.float32)
    for l in range(NL):
        C = _wigner_coeffs(l)
        B = _real_basis(l)
        Cr = np.real(np.einsum('ua,abq,vb->uvq', B, C, B.conj())).astype(np.float32)
        n = 2 * l + 1
        CC[OFF49[l]:OFF49[l] + n, OFF455[l]:OFF455[l] + n * n] = \
            np.moveaxis(Cr, 2, 0).reshape(n, n * n)
        for ul in range(n):
            m = ul - l
            v = OFF49[l] + ul
            M14[abs(m), v * R:(v + 1) * R] = 1.0
            if m != 0:
                M14[abs(m), 490 + v * R:490 + (v + 1) * R] = float(np.sign(m))
    return CC, M14


def _flip49(x):
    out = np.empty_like(x)
    for l in range(NL):
        out[OFF49[l]:OFF49[l + 1]] = x[OFF49[l]:OFF49[l + 1]][::-1]
    return out


def _ki(d, s):
    for k, ss in TAPS[d]:
        if ss == s:
            return k
    return None


def _sis(d):
    return [s for s in (-1, 0, 1) if _ki(d, s) is not None]


def _prep_weights(item_rep, W, b, k1, b1, k2, b2, k3, b3, k4, b4):
    bf16 = ml_dtypes.bfloat16
    inp = {}
    inp['repM'] = np.concatenate(
        [np.tile(item_rep.reshape(-1), (7, 1)),
         np.tile(_flip49(item_rep).reshape(-1), (7, 1))], 1).astype(np.float32)
    CC, M14 = _build_consts()
    inp['CC'], inp['M14'] = CC, M14
    inp['ident'] = np.eye(128, dtype=np.float32)
    Wp = np.zeros((512, 4096), np.float32)
    Wp[:490] = W
    Wp[490] = b  # bias row; itemT row 490 is set to ones
    inp['Wb'] = np.ascontiguousarray(Wp.reshape(4, 128, 4096)).astype(bf16)
    k1s = np.zeros((128, 4, 4, 2, 128), np.float32)
    for pi4, (di, dj) in enumerate([(0, 0), (0, 1), (1, 0), (1, 1)]):
        tn = 0
        for si in _sis(di):
            for sj in _sis(dj):
                kk = k1[_ki(di, si), _ki(dj, sj)]
                for h in range(2):
                    k1s[:, pi4, tn, h, :] = kk[h * 128:(h + 1) * 128]
                tn += 1
    inp['k1s'] = k1s.astype(bf16)
    inp['b1'] = b1.reshape(128, 1).astype(np.float32)
    # conv2 col-tiled: lhsT [cin128, di, ri, t, (dj,ch,co32)]
    k2n = np.zeros((128, 2, 2, 2, 128), np.float32)
    for di in range(2):
        for ri in range(2):
            for tt in range(2):
                for dj in range(2):
                    kk = k2[_ki(di, di - 1 + ri), _ki(dj, dj - 1 + tt)]
                    k2n[:, di, ri, tt, dj * 64:(dj + 1) * 64] = kk
    inp['k2n'] = k2n.astype(bf16)
    inp['b2'] = np.tile(b2, 2).reshape(128, 1).astype(np.float32)
    # conv3 col-tiled: lhsT [(rsh,cin64)128, di, t, (jm=(2*qloc+dj),co32)]
    k3n = np.zeros((2, 64, 2, 2, 4, 32), np.float32)
    for di in range(2):
        for rsh in range(2):
            for tt in range(2):
                for qloc in range(2):
                    for dj in range(2):
                        k3n[rsh, :, di, tt, 2 * qloc + dj, :] = \
                            k3[_ki(di, di - 1 + rsh), _ki(dj, dj - 1 + tt)]
    inp['k3n'] = k3n.reshape(128, 2, 2, 128).astype(bf16)
    inp['b3'] = np.tile(b3, 4).reshape(128, 1).astype(np.float32)
    # conv4 col-tiled banded: tile t4=(di,jh) covers m=di*64+jh*32+2*(jout-16jh)+dj
    # K=(jm,c32) block window; 5 blocks per tile, 2 exact row steps
    k4n = np.zeros((4, 32, 4, 2, 5, 32), np.float32)
    for t4 in range(4):
        di, jh = t4 >> 1, t4 & 1
        for ri in range(2):
            ki = _ki(di, di - 1 + ri)
            for bb in range(5):
                for jm in range(4):
                    j = 4 * (bb + 3 * jh) + jm
                    for mloc in range(32):
                        jout = 16 * jh + (mloc >> 1)
                        dj = mloc & 1
                        sj = j - jout
                        if sj in (dj - 1, dj):
                            k4n[jm, :, t4, ri, bb, mloc] = k4[ki, _ki(dj, sj), :, 0]
    inp['k4n'] = k4n.reshape(128, 4, 2, 5, 32).astype(bf16)
    inp['b4'] = np.full((128, 1), float(b4[0]), np.float32)
    return inp


def _build():
    import concourse.bass as bass
    import concourse.mybir as mybir
    import concourse.tile as tile
    from concourse import bacc
    import contextlib

    dt = mybir.dt
    AF = mybir.ActivationFunctionType
    ALU = mybir.AluOpType
    f32, f32r, bf16 = dt.float32, dt.float32r, dt.bfloat16
    nc = bacc.Bacc("TRN2", target_bir_lowering=False, debug=False, num_devices=NCORES)

    def din(name, shape, dtype=f32):
        return nc.dram_tensor(name, list(shape), dtype, kind="ExternalInput").ap()

    ang = din('angles', [NPC, 3])
    repM_d = din('repM', [7, 980])
    CC_d = din('CC', [49, 456], f32r)
    M14_d = din('M14', [7, 980])
    id_d = din('ident', [128, 128])
    Wb_d = din('Wb', [4, 128, 4096], bf16)
    k1s_d = din('k1s', [128, 4, 4, 2, 128], bf16)
    b1_d = din('b1', [128, 1])
    k2n_d = din('k2n', [128, 2, 2, 2, 128], bf16)
    b2_d = din('b2', [128, 1])
    k3n_d = din('k3n', [128, 2, 2, 128], bf16)
    b3_d = din('b3', [128, 1])
    k4n_d = din('k4n', [128, 4, 2, 5, 32], bf16)
    b4_d = din('b4', [128, 1])
    out_d = nc.dram_tensor('out', [NPC, 64, 64], f32, kind="ExternalOutput").ap()

    def mk(t, off, dims):
        a = t[:]
        return bass.AP(tensor=a.tensor, offset=a.offset + off,
                       ap=[[a.ap[0][0], a.ap[0][1]]] + [[s, c] for s, c in dims])

    def mkp(t, p0, pn, off, dims):
        a = t[:]
        return bass.AP(tensor=a.tensor, offset=a.offset + p0 * a.ap[0][0] + off,
                       ap=[[a.ap[0][0], pn]] + [[s, c] for s, c in dims])

    with tile.TileContext(nc) as tc:
        ctx = contextlib.ExitStack()
        wp = ctx.enter_context(tc.tile_pool(name="wts", bufs=1))
        apl = ctx.enter_context(tc.tile_pool(name="acts", bufs=1))
        tp = ctx.enter_context(tc.tile_pool(name="tmp", bufs=1))
        pp = ctx.enter_context(tc.tile_pool(name="ps", bufs=8, space="PSUM"))

        def psum():
            return pp.tile([128, 512], f32, tag="ps", name="ps")

        def load(dram_ap, shape, dtype=f32, tag=None):
            t = wp.tile(shape, dtype, tag=tag)
            nc.sync.dma_start(out=t[:], in_=dram_ap)
            return t

        a3t = []
        for s in range(2):
            at = wp.tile([128, 3], f32, tag=f"a3_{s}", name=f"a3_{s}")
            nc.sync.dma_start(out=at[:], in_=ang[s * 128:(s + 1) * 128, :])
            a3t.append(at)
        # y-stage weights, split so matmul lhsT/rhs share base partition 0
        wdp = tc.tile_pool(name="wdense", bufs=1)
        wdpo = wdp.__enter__()
        def loadw(dram_ap, shape, dtype=f32, tag=None):
            t = wdpo.tile(shape, dtype, tag=tag, name=tag)
            nc.sync.dma_start(out=t[:], in_=dram_ap)
            return t
        repM = loadw(repM_d[:, :], [7, 980], tag="repM")
        CC = loadw(CC_d[:, :], [49, 456], f32r, tag="CC")
        M14 = loadw(M14_d[:, :], [7, 980], tag="M14")
        Fcs = wdpo.tile([7, 980], f32r, tag="Fcs", name="Fcs")
        nc.vector.tensor_mul(Fcs[:], M14[:], repM[:])
        ident = load(id_d[:, :], [128, 128], tag="ident")
        hw = {}

        def load_heavy():
            # deferred so startup DMA doesn't gate the batch-0 wigner compute
            hw['k1s'] = load(k1s_d[:, :, :, :, :], [128, 4, 4, 2, 128], bf16, tag="k1s")
            hw['b1'] = load(b1_d[:, :], [128, 1], tag="b1")
            hw['k2n'] = load(k2n_d[:, :, :, :, :], [128, 2, 2, 2, 128], bf16, tag="k2n")
            hw['b2'] = load(b2_d[:, :], [128, 1], tag="b2")
            hw['k3n'] = load(k3n_d[:, :, :, :], [128, 2, 2, 128], bf16, tag="k3n")
            hw['b3'] = load(b3_d[:, :], [128, 1], tag="b3")
            hw['k4n'] = load(k4n_d[:, :, :, :, :], [128, 4, 2, 5, 32], bf16, tag="k4n")
            hw['b4'] = load(b4_d[:, :], [128, 1], tag="b4")

        c_half = wp.tile([128, 1], f32, tag="c_half", name="c_half")
        c_nhpi = wp.tile([128, 1], f32, tag="c_nhpi", name="c_nhpi")
        c_none = wp.tile([128, 1], f32, tag="c_none", name="c_none")
        c_one = wp.tile([128, 1], f32, tag="c_one", name="c_one")
        nc.vector.memset(c_half[:], 0.5)
        nc.vector.memset(c_nhpi[:], -PI / 2.0)
        nc.vector.memset(c_none[:], -1.0)
        nc.vector.memset(c_one[:], 1.0)
        c_zero = wp.tile([128, 1], f32, tag="c_zero", name="c_zero")
        nc.vector.memset(c_zero[:], 0.0)



        itemTb = [wdpo.tile([128, 256], bf16, tag=f"itemTb{kc}", name=f"itemTb{kc}") for kc in range(4)]
        nc.vector.memset(itemTb[3][:], 0.0)
        Wk = []
        for kc in range(4):
            wt = wdpo.tile([128, 4096], bf16, tag=f"Wk{kc}", name=f"Wk{kc}")
            nc.sync.dma_start(out=wt[:], in_=Wb_d[kc, :, :])
            Wk.append(wt)

        c1in = [apl.tile([128, NPC, 6, 6], bf16, tag=f"c1in{h}", name=f"c1in{h}") for h in range(2)]
        c1o = apl.tile([128, NT, 10, 10], bf16, tag="c1o", name="c1o")
        c2o2 = apl.tile([128, NT, 18, 2, 10], bf16, tag="c2o2", name="c2o2")
        c3o = apl.tile([128, NT, 8, 34], bf16, tag="c3o", name="c3o")
        ot2 = [apl.tile([128, NT, 32], f32, tag=f"otile{q}", name=f"otile{q}")
               for q in range(2)]
        obuf = apl.tile([128, 8, 128], f32, tag="obuf", name="obuf")
        for h in range(2):
            nc.gpsimd.memset(c1in[h][:], 0.0)
        nc.gpsimd.memset(c1o[:].bitcast(f32), 0.0)
        nc.gpsimd.memset(c2o2[:].bitcast(f32), 0.0)
        nc.gpsimd.memset(c3o[:].bitcast(f32), 0.0)

        # ================= Wigner stage (one 128-sample batch) =================
        def wigner_a(s):
            a3 = a3t[s]
            # sx = sin(t/2 - pi/2), cx = sqrt(1 - sx^2)  for t = a, b, g
            sx = tp.tile([128, 3], f32, tag="sx", name="sx")
            cx = tp.tile([128, 3], f32, tag="cx", name="cx")
            sq = tp.tile([128, 3], f32, tag="sqt", name="sqt")
            nc.scalar.activation(sx[:], a3[:], AF.Sin, bias=c_nhpi[:], scale=c_half[:])
            nc.vector.tensor_mul(sq[:], sx[:], sx[:])
            nc.scalar.activation(cx[:], sq[:], AF.Sqrt, bias=c_one[:], scale=c_none[:])
            # half-angle of b: cb = -sx[:,1], sb = cx[:,1]
            cb = tp.tile([128, 1], f32, tag="cb", name="cb")
            sb = tp.tile([128, 1], f32, tag="sb", name="sb")
            nc.vector.tensor_scalar_mul(cb[:], sx[:, 1:2], -1.0)
            nc.vector.tensor_copy(sb[:], cx[:, 1:2])
            # full-angle cos/sin for a, g via double angle: cu = -sx, su = cx
            # c1 = 2 cu^2 - 1 ; s1 = 2 su cu
            cosT = tp.tile([128, 14], f32, tag=f"cosT{s}", name=f"cosT{s}")  # cols 0..6 cos(m a), 7..13 cos(m g)
            sinT = tp.tile([128, 14], f32, tag=f"sinT{s}", name=f"sinT{s}")
            nc.vector.memset(cosT[:, 0:1], 1.0)
            nc.vector.memset(cosT[:, 7:8], 1.0)
            nc.vector.memset(sinT[:, 0:1], 0.0)
            nc.vector.memset(sinT[:, 7:8], 0.0)
            cu = tp.tile([128, 2], f32, tag="cu", name="cu")
            su = tp.tile([128, 2], f32, tag="su", name="su")
            nc.vector.tensor_copy(cu[:], mk(sx, 0, [(2, 2)]))   # sx cols (a, g)
            nc.vector.tensor_copy(su[:], mk(cx, 0, [(2, 2)]))
            t0 = tp.tile([128, 2], f32, tag="t0", name="t0")
            nc.vector.tensor_mul(t0[:], cu[:], cu[:])
            c1 = tp.tile([128, 2], f32, tag="c1", name="c1")
            s1 = tp.tile([128, 2], f32, tag="s1", name="s1")
            nc.vector.tensor_scalar(c1[:], t0[:], 2.0, -1.0, op0=ALU.mult, op1=ALU.add)
            nc.vector.tensor_mul(t0[:], su[:], cu[:])
            nc.vector.tensor_scalar_mul(s1[:], t0[:], 2.0)
            # note cu = -sx is cos(t/2)? no: cu here = sx = -cos(t/2); but
            # c1 = 2 cu^2 - 1 = 2 cos^2(t/2) - 1 = cos(t) regardless of sign.
            # s1 = 2 su cu = 2 sin(t/2) (-cos(t/2)) = -sin(t) -> fix sign:
            nc.vector.tensor_scalar_mul(s1[:], s1[:], -1.0)
            cm = [None, (c1, 0)]
            # write m=1
            nc.vector.tensor_copy(mk(cosT, 1, [(7, 2)]), c1[:])
            nc.vector.tensor_copy(mk(sinT, 1, [(7, 2)]), s1[:])
            ta = tp.tile([128, 2], f32, tag="ta", name="ta")
            tb = tp.tile([128, 2], f32, tag="tb", name="tb")
            for m in range(2, 7):
                pcm = mk(cosT, m - 1, [(7, 2)])
                psm = mk(sinT, m - 1, [(7, 2)])
                nc.vector.tensor_mul(ta[:], pcm, c1[:])
                nc.vector.tensor_mul(tb[:], psm, s1[:])
                nc.vector.tensor_sub(mk(cosT, m, [(7, 2)]), ta[:], tb[:])
                nc.vector.tensor_mul(ta[:], psm, c1[:])
                nc.vector.tensor_mul(tb[:], pcm, s1[:])
                nc.vector.tensor_add(mk(sinT, m, [(7, 2)]), ta[:], tb[:])
            # power tables
            cpow = tp.tile([128, 13], f32, tag="cpow", name="cpow")
            spow = tp.tile([128, 13], f32, tag="spow", name="spow")
            for pw, base in ((cpow, cb), (spow, sb)):
                nc.vector.memset(pw[:, 0:1], 1.0)
                nc.vector.tensor_copy(pw[:, 1:2], base[:])
                xw = tp.tile([128, 1], f32, tag="xw", name="xw")
                nc.vector.tensor_mul(xw[:], base[:], base[:])
                nc.vector.tensor_scalar_mul(pw[:, 2:4], pw[:, 0:2], xw[:])
                nc.vector.tensor_mul(xw[:], xw[:], xw[:])
                nc.vector.tensor_scalar_mul(pw[:, 4:8], pw[:, 0:4], xw[:])
                nc.vector.tensor_mul(xw[:], xw[:], xw[:])
                nc.vector.tensor_scalar_mul(pw[:, 8:13], pw[:, 0:5], xw[:])
            # feat [128, 77] = [P49 | cosT 14 | sinT 14]
            feat = tp.tile([128, 103], f32, tag="feat", name="feat")
            for l in range(NL):
                n = 2 * l + 1
                nc.vector.tensor_mul(feat[:, OFF49[l]:OFF49[l] + n],
                                     mk(cpow, 2 * l, [(-1, n)]), spow[:, 0:n])
            nc.vector.tensor_copy(feat[:, 64:71], cosT[:, 7:14])
            nc.vector.tensor_copy(feat[:, 96:103], sinT[:, 7:14])
            # transpose -> base-0 lhsT tiles
            ptp = psum()
            nc.tensor.transpose(ptp[0:103, 0:128], feat[:, 0:103], ident[:])
            featP = tp.tile([49, 128], f32r, tag="featP", name="featP")
            featCG = tp.tile([7, 128], f32r, tag="featCG", name="featCG")
            featSG = tp.tile([7, 128], f32r, tag="featSG", name="featSG")
            nc.vector.tensor_copy(featP[:], ptp[0:49, 0:128])
            nc.vector.tensor_copy(featCG[:], ptp[64:71, 0:128])
            nc.vector.tensor_copy(featSG[:], ptp[96:103, 0:128])
            # dvec = P_all @ CC : [128, 455]
            pd = psum()
            nc.tensor.matmul(pd[:, 0:456], featP[:],
                             CC[:], start=True, stop=True)

            # y = T_g @ F : [128, 490]
            py = psum()
            nc.tensor.matmul(py[:, 0:490], featCG[:],
                             Fcs[:, 0:490], start=True, stop=False)
            nc.tensor.matmul(py[:, 0:490], featSG[:],
                             Fcs[:, 490:980], start=False, stop=True)
            # evacuate psums here so none are held across interleaved conv tiles
            y = tp.tile([128, 490], bf16, tag="y", name="y")
            nc.scalar.activation(y[:], py[:, 0:490], AF.Identity, bias=c_zero[:])
            dvec = tp.tile([128, 456], bf16, tag="dv", name="dvec")
            nc.scalar.activation(dvec[:], pd[:, 0:456], AF.Identity, bias=c_zero[:])
            return y, dvec, cosT, sinT

        def wigner_b(s, y, dvec, cosT, sinT):
            # t2[n,(u,r)] = sum_v d[n,(u,v)] y[n,(v,r)] via wide mul + X-axis reduce
            t2 = tp.tile([128, 490], f32, tag="t2", name="t2")
            prod = tp.tile([128, 1690], bf16, tag="prod", name="prod")
            nc.vector.tensor_mul(t2[:, 0:R], y[:, 0:R], mk(dvec, 0, [(0, R)]))
            for l in range(1, NL):
                n = 2 * l + 1
                d_ap = mk(dvec, OFF455[l], [(n, n), (0, R), (1, n)])
                y_ap = mk(y, OFF49[l] * R, [(0, n), (1, R), (R, n)])
                nc.vector.tensor_mul(mk(prod, 0, [(n * R, n), (n, R), (1, n)]),
                                     d_ap, y_ap)
                nc.vector.tensor_reduce(
                    mk(t2, OFF49[l] * R, [(R, n), (1, R)]),
                    mk(prod, 0, [(n * R, n), (n, R), (1, n)]),
                    axis=mybir.AxisListType.X, op=ALU.add)
            # Za apply: item = caE*t2 + saE*flip(t2)
            nsin = tp.tile([128, 7], f32, tag="nsin", name="nsin")
            nc.scalar.activation(nsin[:], sinT[:, 0:7], AF.Identity,
                                 bias=c_zero[:], scale=c_none[:])
            caE = tp.tile([128, 490], f32, tag="caE", name="caE")
            saE = tp.tile([128, 490], f32, tag="saE", name="saE")
            for l in range(NL):
                n = 2 * l + 1
                base = OFF49[l] * R
                nc.scalar.activation(caE[:, base:base + (l + 1) * R],
                                     mk(cosT, l, [(-1, l + 1), (0, R)]),
                                     AF.Identity, bias=c_zero[:])
                nc.scalar.activation(saE[:, base:base + (l + 1) * R],
                                     mk(nsin, l, [(-1, l + 1), (0, R)]),
                                     AF.Identity, bias=c_zero[:])
                if l > 0:
                    nc.scalar.activation(caE[:, base + l * R:base + n * R],
                                         mk(cosT, 0, [(1, l + 1), (0, R)]),
                                         AF.Identity, bias=c_zero[:])
                    nc.scalar.activation(saE[:, base + l * R:base + n * R],
                                         mk(sinT, 0, [(1, l + 1), (0, R)]),
                                         AF.Identity, bias=c_zero[:])
            item = tp.tile([128, 496], f32, tag="item", name="item")
            tmp2 = tp.tile([128, 490], f32, tag="tmpf", name="tmpf")
            nc.vector.memset(item[:, 490:491], 1.0)  # ones col -> dense bias row
            nc.vector.tensor_mul(item[:, 0:490], caE[:], t2[:])
            for l in range(NL):
                n = 2 * l + 1
                base = OFF49[l] * R
                nc.vector.tensor_mul(tmp2[:, base:base + n * R], saE[:, base:base + n * R],
                                     mk(t2, base + (n - 1) * R, [(-R, n), (1, R)]))
            nc.vector.tensor_add(item[:, 0:490], item[:, 0:490], tmp2[:])
            for kc in range(4):
                cnt = 128 if kc < 3 else 107
                pit = psum()
                nc.tensor.transpose(pit[0:cnt, 0:128], item[:, kc * 128:kc * 128 + cnt],
                                    ident[:])
                nc.vector.tensor_copy(itemTb[kc][0:cnt, s * 128:(s + 1) * 128],
                                      pit[0:cnt, 0:128])

        # ================= dense -> c1in [(c-half),(n,6,6)] bf16 =================
        def dense_half(s):
            for yy in range(4):
                for h in range(2):
                    pd2 = psum()
                    for xx in range(4):
                        mc = yy * 8 + xx * 2 + h
                        for kc in range(4):
                            nc.tensor.matmul(pd2[:, xx * 128:(xx + 1) * 128],
                                             Wk[kc][:, mc * 128:(mc + 1) * 128],
                                             itemTb[kc][:, s * 128:(s + 1) * 128],
                                             start=(kc == 0), stop=(kc == 3))
                    srcap = bass.AP(tensor=pd2[:].tensor, offset=pd2[:].offset,
                                    ap=[list(pd2[:].ap[0]), [1, 128], [128, 4]])
                    nc.scalar.activation(mk(c1in[h], (1 + yy) * 6 + 1 + s * 128 * 36,
                                            [(36, 128), (1, 4)]),
                                         srcap, AF.Relu, bias=c_zero[:])

        # ================= conv stack, one n-tile of 32 =================
        def conv_tile(t):
            ns = t * NT
            # conv1: psum [c128, (n32,4,4)]
            for pi4, (di, dj) in enumerate([(0, 0), (0, 1), (1, 0), (1, 1)]):
                ps = psum()
                tn = 0
                for si in _sis(di):
                    for sj in _sis(dj):
                        for h in range(2):
                            rhs = mk(c1in[h], ns * 36 + (1 + si) * 6 + (1 + sj),
                                     [(36, NT), (6, 4), (1, 4)])
                            nc.tensor.matmul(ps[:, 0:512], k1s[:, pi4, tn, h, :], rhs,
                                             start=(tn == 0 and h == 0),
                                             stop=(tn == 3 and h == 1))
                        tn += 1
                dst1 = mk(c1o, (1 + di) * 10 + (1 + dj),
                          [(100, NT), (20, 4), (2, 4)])
                if pi4 % 2 == 0:
                    nc.scalar.activation(dst1, ps[:, 0:512], AF.Relu, bias=b1[:])
                else:
                    nc.vector.tensor_scalar(dst1, ps[:, 0:512], b1[:], 0.0,
                                            op0=ALU.add, op1=ALU.max)
            # conv2 col-tiled: psum [(dj,ch,c32) via 4 col tiles, (n8,p8,pj8)]
            # out rows 2p+di (psum per di); tile j4=(dj,ch); acc (ri,t) exact taps
            for g in range(4):
                for di in range(2):
                    ps = psum()
                    for st, (ri, tt) in enumerate(((0, 0), (1, 0), (0, 1), (1, 1))):
                        for dj in range(2):
                            rhs = mk(c1o, (g * 8) * 100 + (di + ri) * 10 + dj + tt,
                                     [(100, 8), (10, 8), (1, 8)])
                            nc.tensor.matmul(ps[64 * dj:64 * dj + 64, 0:512],
                                             k2n[:, di, ri, tt, 64 * dj:64 * dj + 64],
                                             rhs, start=(st == 0), stop=(st == 3),
                                             tile_position=(0, 64 * dj))
                    # evac rsh0: rows r=2p+di at slot r+1; (dj,pj)-split cols, pj+1
                    for dj in range(2):
                        dst = mkp(c2o2, 0, 64,
                                  (g * 8) * 360 + (di + 1) * 20 + dj * 10 + 1,
                                  [(360, 8), (40, 8), (1, 8)])
                        src = ps[64 * dj:64 * dj + 64, 0:512]
                        bslc = b2[64 * dj:64 * dj + 64, :]
                        if dj == 0:
                            nc.scalar.activation(dst, src, AF.Relu, bias=bslc)
                        else:
                            nc.vector.tensor_scalar(dst, src, bslc, 0.0,
                                                    op0=ALU.add, op1=ALU.max)
                # rsh1 half = rows shifted one slot down, via sbuf->sbuf DMA
                nc.sync.dma_start(
                    out=mkp(c2o2, 64, 64, (g * 8) * 360, [(360, 8), (1, 320)]),
                    in_=mkp(c2o2, 0, 64, (g * 8) * 360 + 20, [(360, 8), (1, 320)]))
            # conv3 col-tiled: psum [(jm4,c32) via 4 col tiles, (n4,p16,qp8)]
            # K=(rsh,c64) covers row taps; acc t covers col taps exactly
            for di in range(2):
                for g in range(8):
                    ps = psum()
                    for tt in range(2):
                        for jm in range(4):
                            qloc, dj = jm >> 1, jm & 1
                            o = qloc + dj - 1 + tt
                            rhs = mk(c2o2, (g * 4) * 360 + di * 20
                                     + (o & 1) * 10 + (o >> 1) + 1,
                                     [(360, 4), (20, 16), (1, 8)])
                            nc.tensor.matmul(ps[32 * jm:32 * jm + 32, 0:512],
                                             k3n[:, di, tt, 32 * jm:32 * jm + 32],
                                             rhs, start=(tt == 0), stop=(tt == 1),
                                             tile_position=(0, 32 * jm))
                    dst3 = mk(c3o, (g * 4) * 272 + 1 + di,
                              [(272, 4), (34, 8), (2, 16)])
                    ps3 = bass.AP(tensor=ps[:].tensor, offset=ps[:].offset,
                                  ap=[list(ps[:].ap[0]), [128, 4], [1, 8], [8, 16]])
                    if g % 2 == 0:
                        nc.vector.tensor_scalar(dst3, ps3, b3[:],
                                                0.0, op0=ALU.add, op1=ALU.max)
                    else:
                        nc.scalar.activation(dst3, ps3, AF.Relu, bias=b3[:])
            # conv4 col-tiled: tile t4=(di,jh); psum m = di*64+jh*32+2*joutloc+dj
            pss4 = [psum() for _ in range(2)]
            for st in range(10):
                bb, ri = st >> 1, st & 1
                for c2c in range(2):
                    for t4 in range(4):
                        di, jh = t4 >> 1, t4 & 1
                        si = di - 1 + ri
                        rhs = mk(c3o, (c2c * 16) * 272 + (bb + 3 * jh) * 34 + 1 + si,
                                 [(272, 16), (1, 32)])
                        nc.tensor.matmul(pss4[c2c][32 * t4:32 * t4 + 32, 0:512],
                                         k4n[:, t4, ri, bb, :], rhs,
                                         start=(st == 0), stop=(st == 9),
                                         tile_position=(0, 32 * t4))
            otile = ot2[t % 2]
            for c2c in range(2):
                dst4 = mk(otile, (c2c * 16) * 32, [(32, 16), (1, 32)])
                if c2c == 0:
                    nc.scalar.activation(dst4, pss4[c2c][:, 0:512], AF.Identity,
                                         bias=b4[:])
                else:
                    nc.vector.tensor_scalar(dst4, pss4[c2c][:, 0:512], b4[:], 0.0,
                                            op0=ALU.add, op1=ALU.add)
        # output stage deferred: transposes queue after next tile's convs
        def conv_out(t):
            ns = t * NT
            otile = ot2[t % 2]
            for cq in range(2):
                pst = psum()
                for cc in range(4):
                    cch = cq * 4 + cc
                    nc.tensor.transpose(pst[0:128, cc * 128:cc * 128 + 128],
                                        otile[:, cch * 4:cch * 4 + 4, :].bitcast(f32),
                                        ident[:])
                nc.scalar.activation(obuf[:, cq * 4:cq * 4 + 4, :], pst[:, 0:512],
                                     AF.Identity, bias=c_zero[:])
            dst = bass.AP(tensor=out_d.tensor, offset=out_d.offset + ns * 4096,
                          ap=[[4096, 4], [128, 32], [16384, 8], [1, 128]])
            nc.sync.dma_start(out=dst, in_=obuf[:])

        # ================= orchestration: overlap wigner(s=1) with convs =================
        ya0 = wigner_a(0)
        wigner_b(0, *ya0)
        load_heavy()
        k1s, b1 = hw['k1s'], hw['b1']
        k2n, b2 = hw['k2n'], hw['b2']
        k3n, b3 = hw['k3n'], hw['b3']
        k4n, b4 = hw['k4n'], hw['b4']
        dense_half(0)
        conv_tile(0)
        ya1 = wigner_a(1)
        conv_tile(1)
        conv_out(0)
        wigner_b(1, *ya1)
        conv_tile(2)
        conv_out(1)
        dense_half(1)
        conv_tile(3)
        conv_out(2)
        for t in range(4, 8):
            conv_tile(t)
            conv_out(t - 1)
        conv_out(7)
        wdp.__exit__(None, None, None)
        ctx.close()
    nc.compile()
    return nc


_NC_CACHE = {}


def kernel(angles, item_rep, W, b, k1, b1, k2, b2, k3, b3, k4, b4):
    _install_axon_shim()
    from concourse.bass_utils import run_bass_kernel_spmd
    if 'nc' not in _NC_CACHE:
        _NC_CACHE['nc'] = _build()
    nc = _NC_CACHE['nc']
    wts = _prep_weights(np.asarray(item_rep, np.float32), np.asarray(W, np.float32),
                        np.asarray(b, np.float32), np.asarray(k1, np.float32),
                        np.asarray(b1, np.float32), np.asarray(k2, np.float32),
                        np.asarray(b2, np.float32), np.asarray(k3, np.float32),
                        np.asarray(b3, np.float32), np.asarray(k4, np.float32),
                        np.asarray(b4, np.float32))
    angles = np.asarray(angles, np.float32)
    in_maps = []
    for c in range(NCORES):
        m = dict(wts)
        m['angles'] = np.ascontiguousarray(angles[c * NPC:(c + 1) * NPC])
        in_maps.append(m)
    res = run_bass_kernel_spmd(nc, in_maps, core_ids=list(range(NCORES)))
    return np.concatenate([r['out'][:, None, :, :] for r in res.results], axis=0)



# revision 40
# speedup vs baseline: 1.2870x; 1.0273x over previous
"""Trainium2 Bass kernel for nn_ActionNet (Wigner-D block-diag rotation + dense +
4x stride-2 conv_transpose decoder), data-parallel over 8 NeuronCores.

Math: real Wigner D^l(a,b,g) = Zr(a) @ dr(b) @ Zr(g), with
  Zr(t): Zr[m,m]=cos(mt), Zr[l+m,l-m]=sin(mt), Zr[l-m,l+m]=-sin(mt)
  dr(b)[u,v] = sum_q Cr_l[u,v,q] cos(b/2)^(2l-q) sin(b/2)^q,  Cr_l = Re(B C_l B^H)
conv_transpose(s=2,k=4,SAME) phases (verified vs jax):
  out[2p+d]: d=0 -> K[2] x[p] + K[0] x[p-1];  d=1 -> K[1] x[p] + K[3] x[p+1]
Trig: sx = sin(t/2 - pi/2) (safe LUT domain), cx = sqrt(1-sx^2);
  cos(t/2) = -sx, sin(t/2) = cx; then double-angle + recurrence for cos/sin(m t).
"""
import math
import sys
import types

import numpy as np

sys.path.insert(0, '/opt/trn_rl_repo')
import ml_dtypes

DEGREES = 6
NL = DEGREES + 1
R = 10
N_BATCH = 2048
NCORES = 8
NPC = N_BATCH // NCORES
NT = 32
PI = math.pi
TAPS = {0: [(2, 0), (0, -1)], 1: [(1, 0), (3, 1)]}
OFF49 = [l * l for l in range(NL + 1)]
OFF455 = np.cumsum([0] + [(2 * l + 1) ** 2 for l in range(NL)]).tolist()


def _install_axon_shim():
    if 'antenv.axon_hooks' in sys.modules:
        return
    mod = types.ModuleType('antenv.axon_hooks')
    _h = [None]
    mod.set_axon_ntff_profile_hook = lambda h: _h.__setitem__(0, h)
    mod.get_axon_ntff_profile_hook = lambda: _h[0]
    sys.modules['antenv.axon_hooks'] = mod
    try:
        import antenv
        antenv.axon_hooks = mod
        from trn_agent_boot.trn_boot import _ntff_profile_via_ctypes
        mod.set_axon_ntff_profile_hook(_ntff_profile_via_ctypes('/opt/axon/libaxon_pjrt.so'))
    except Exception:
        pass


def _wigner_coeffs(l):
    f = math.factorial
    n = 2 * l + 1
    C = np.zeros((n, n, n))
    for mp in range(-l, l + 1):
        for m in range(-l, l + 1):
            pref = math.sqrt(f(l + mp) * f(l - mp) * f(l + m) * f(l - m))
            for s in range(max(0, m - mp), min(l + m, l - mp) + 1):
                q = mp - m + 2 * s
                den = f(l + m - s) * f(s) * f(mp - m + s) * f(l - mp - s)
                C[mp + l, m + l, q] += ((-1.0) ** (mp - m + s)) * pref / den
    return C


def _real_basis(l):
    n = 2 * l + 1
    B = np.zeros((n, n), dtype=np.complex128)
    B[l, l] = 1.0
    isq = 1.0 / math.sqrt(2.0)
    for m in range(1, l + 1):
        B[l + m, l + m] = ((-1) ** m) * isq
        B[l + m, l - m] = isq
        B[l - m, l - m] = 1j * isq
        B[l - m, l + m] = -1j * ((-1) ** m) * isq
    return B


def _build_consts():
    CC = np.zeros((49, 456), np.float32)
    M14 = np.zeros((7, 980), np.float32)
    for l in range(NL):
        C = _wigner_coeffs(l)
        B = _real_basis(l)
        Cr = np.real(np.einsum('ua,abq,vb->uvq', B, C, B.conj())).astype(np.float32)
        n = 2 * l + 1
        CC[OFF49[l]:OFF49[l] + n, OFF455[l]:OFF455[l] + n * n] = \
            np.moveaxis(Cr, 2, 0).reshape(n, n * n)
        for ul in range(n):
            m = ul - l
            v = OFF49[l] + ul
            M14[abs(m), v * R:(v + 1) * R] = 1.0
            if m != 0:
                M14[abs(m), 490 + v * R:490 + (v + 1) * R] = float(np.sign(m))
    return CC, M14


def _flip49(x):
    out = np.empty_like(x)
    for l in range(NL):
        out[OFF49[l]:OFF49[l + 1]] = x[OFF49[l]:OFF49[l + 1]][::-1]
    return out


def _ki(d, s):
    for k, ss in TAPS[d]:
        if ss == s:
            return k
    return None


def _sis(d):
    return [s for s in (-1, 0, 1) if _ki(d, s) is not None]


def _prep_weights(item_rep, W, b, k1, b1, k2, b2, k3, b3, k4, b4):
    bf16 = ml_dtypes.bfloat16
    inp = {}
    inp['repM'] = np.concatenate(
        [np.tile(item_rep.reshape(-1), (7, 1)),
         np.tile(_flip49(item_rep).reshape(-1), (7, 1))], 1).astype(np.float32)
    CC, M14 = _build_consts()
    inp['CC'], inp['M14'] = CC, M14
    inp['ident'] = np.eye(128, dtype=np.float32)
    Wp = np.zeros((512, 4096), np.float32)
    Wp[:490] = W
    Wp[490] = b  # bias row; itemT row 490 is set to ones
    inp['Wb'] = np.ascontiguousarray(Wp.reshape(4, 128, 4096)).astype(bf16)
    k1s = np.zeros((128, 4, 4, 2, 128), np.float32)
    for pi4, (di, dj) in enumerate([(0, 0), (0, 1), (1, 0), (1, 1)]):
        tn = 0
        for si in _sis(di):
            for sj in _sis(dj):
                kk = k1[_ki(di, si), _ki(dj, sj)]
                for h in range(2):
                    k1s[:, pi4, tn, h, :] = kk[h * 128:(h + 1) * 128]
                tn += 1
    inp['k1s'] = k1s.astype(bf16)
    inp['b1'] = b1.reshape(128, 1).astype(np.float32)
    # conv2 col-tiled: lhsT [cin128, di, ri, t, (dj,ch,co32)]
    k2n = np.zeros((128, 2, 2, 2, 128), np.float32)
    for di in range(2):
        for ri in range(2):
            for tt in range(2):
                for dj in range(2):
                    kk = k2[_ki(di, di - 1 + ri), _ki(dj, dj - 1 + tt)]
                    k2n[:, di, ri, tt, dj * 64:(dj + 1) * 64] = kk
    inp['k2n'] = k2n.astype(bf16)
    inp['b2'] = np.tile(b2, 2).reshape(128, 1).astype(np.float32)
    # conv3 col-tiled: lhsT [(rsh,cin64)128, di, t, (jm=(2*qloc+dj),co32)]
    k3n = np.zeros((2, 64, 2, 2, 4, 32), np.float32)
    for di in range(2):
        for rsh in range(2):
            for tt in range(2):
                for qloc in range(2):
                    for dj in range(2):
                        k3n[rsh, :, di, tt, 2 * qloc + dj, :] = \
                            k3[_ki(di, di - 1 + rsh), _ki(dj, dj - 1 + tt)]
    inp['k3n'] = k3n.reshape(128, 2, 2, 128).astype(bf16)
    inp['b3'] = np.tile(b3, 4).reshape(128, 1).astype(np.float32)
    # conv4 col-tiled banded: tile t4=(di,jh) covers m=di*64+jh*32+2*(jout-16jh)+dj
    # K=(jm,c32) block window; 5 blocks per tile, 2 exact row steps
    k4n = np.zeros((4, 32, 4, 2, 5, 32), np.float32)
    for t4 in range(4):
        di, jh = t4 >> 1, t4 & 1
        for ri in range(2):
            ki = _ki(di, di - 1 + ri)
            for bb in range(5):
                for jm in range(4):
                    j = 4 * (bb + 3 * jh) + jm
                    for mloc in range(32):
                        jout = 16 * jh + (mloc >> 1)
                        dj = mloc & 1
                        sj = j - jout
                        if sj in (dj - 1, dj):
                            k4n[jm, :, t4, ri, bb, mloc] = k4[ki, _ki(dj, sj), :, 0]
    inp['k4n'] = k4n.reshape(128, 4, 2, 5, 32).astype(bf16)
    inp['b4'] = np.full((128, 1), float(b4[0]), np.float32)
    return inp


def _build():
    import concourse.bass as bass
    import concourse.mybir as mybir
    import concourse.tile as tile
    from concourse import bacc
    import contextlib

    dt = mybir.dt
    AF = mybir.ActivationFunctionType
    ALU = mybir.AluOpType
    f32, f32r, bf16 = dt.float32, dt.float32r, dt.bfloat16
    nc = bacc.Bacc("TRN2", target_bir_lowering=False, debug=False, num_devices=NCORES)

    def din(name, shape, dtype=f32):
        return nc.dram_tensor(name, list(shape), dtype, kind="ExternalInput").ap()

    ang = din('angles', [NPC, 3])
    repM_d = din('repM', [7, 980])
    CC_d = din('CC', [49, 456], f32r)
    M14_d = din('M14', [7, 980])
    id_d = din('ident', [128, 128])
    Wb_d = din('Wb', [4, 128, 4096], bf16)
    k1s_d = din('k1s', [128, 4, 4, 2, 128], bf16)
    b1_d = din('b1', [128, 1])
    k2n_d = din('k2n', [128, 2, 2, 2, 128], bf16)
    b2_d = din('b2', [128, 1])
    k3n_d = din('k3n', [128, 2, 2, 128], bf16)
    b3_d = din('b3', [128, 1])
    k4n_d = din('k4n', [128, 4, 2, 5, 32], bf16)
    b4_d = din('b4', [128, 1])
    out_d = nc.dram_tensor('out', [NPC, 64, 64], f32, kind="ExternalOutput").ap()

    def mk(t, off, dims):
        a = t[:]
        return bass.AP(tensor=a.tensor, offset=a.offset + off,
                       ap=[[a.ap[0][0], a.ap[0][1]]] + [[s, c] for s, c in dims])

    def mkp(t, p0, pn, off, dims):
        a = t[:]
        return bass.AP(tensor=a.tensor, offset=a.offset + p0 * a.ap[0][0] + off,
                       ap=[[a.ap[0][0], pn]] + [[s, c] for s, c in dims])

    with tile.TileContext(nc) as tc:
        ctx = contextlib.ExitStack()
        wp = ctx.enter_context(tc.tile_pool(name="wts", bufs=1))
        apl = ctx.enter_context(tc.tile_pool(name="acts", bufs=1))
        tp = ctx.enter_context(tc.tile_pool(name="tmp", bufs=1))
        pp = ctx.enter_context(tc.tile_pool(name="ps", bufs=8, space="PSUM"))

        def psum():
            return pp.tile([128, 512], f32, tag="ps", name="ps")

        def load(dram_ap, shape, dtype=f32, tag=None):
            t = wp.tile(shape, dtype, tag=tag)
            nc.sync.dma_start(out=t[:], in_=dram_ap)
            return t

        a3t = []
        for s in range(2):
            at = wp.tile([128, 3], f32, tag=f"a3_{s}", name=f"a3_{s}")
            nc.sync.dma_start(out=at[:], in_=ang[s * 128:(s + 1) * 128, :])
            a3t.append(at)
        # y-stage weights, split so matmul lhsT/rhs share base partition 0
        wdp = tc.tile_pool(name="wdense", bufs=1)
        wdpo = wdp.__enter__()
        def loadw(dram_ap, shape, dtype=f32, tag=None):
            t = wdpo.tile(shape, dtype, tag=tag, name=tag)
            nc.sync.dma_start(out=t[:], in_=dram_ap)
            return t
        repM = loadw(repM_d[:, :], [7, 980], tag="repM")
        CC = loadw(CC_d[:, :], [49, 456], f32r, tag="CC")
        M14 = loadw(M14_d[:, :], [7, 980], tag="M14")
        Fcs = wdpo.tile([7, 980], f32r, tag="Fcs", name="Fcs")
        nc.vector.tensor_mul(Fcs[:], M14[:], repM[:])
        ident = load(id_d[:, :], [128, 128], tag="ident")
        hw = {}

        def load_heavy():
            # deferred so startup DMA doesn't gate the batch-0 wigner compute
            hw['k1s'] = load(k1s_d[:, :, :, :, :], [128, 4, 4, 2, 128], bf16, tag="k1s")
            hw['b1'] = load(b1_d[:, :], [128, 1], tag="b1")
            hw['k2n'] = load(k2n_d[:, :, :, :, :], [128, 2, 2, 2, 128], bf16, tag="k2n")
            hw['b2'] = load(b2_d[:, :], [128, 1], tag="b2")
            hw['k3n'] = load(k3n_d[:, :, :, :], [128, 2, 2, 128], bf16, tag="k3n")
            hw['b3'] = load(b3_d[:, :], [128, 1], tag="b3")
            hw['k4n'] = load(k4n_d[:, :, :, :, :], [128, 4, 2, 5, 32], bf16, tag="k4n")
            hw['b4'] = load(b4_d[:, :], [128, 1], tag="b4")

        c_half = wp.tile([128, 1], f32, tag="c_half", name="c_half")
        c_nhpi = wp.tile([128, 1], f32, tag="c_nhpi", name="c_nhpi")
        c_none = wp.tile([128, 1], f32, tag="c_none", name="c_none")
        c_one = wp.tile([128, 1], f32, tag="c_one", name="c_one")
        nc.vector.memset(c_half[:], 0.5)
        nc.vector.memset(c_nhpi[:], -PI / 2.0)
        nc.vector.memset(c_none[:], -1.0)
        nc.vector.memset(c_one[:], 1.0)
        c_zero = wp.tile([128, 1], f32, tag="c_zero", name="c_zero")
        nc.vector.memset(c_zero[:], 0.0)



        itemTb = [wdpo.tile([128, 256], bf16, tag=f"itemTb{kc}", name=f"itemTb{kc}") for kc in range(4)]
        nc.vector.memset(itemTb[3][:], 0.0)
        Wk = []
        for kc in range(4):
            wt = wdpo.tile([128, 4096], bf16, tag=f"Wk{kc}", name=f"Wk{kc}")
            nc.sync.dma_start(out=wt[:], in_=Wb_d[kc, :, :])
            Wk.append(wt)

        c1in = [apl.tile([128, NPC, 6, 6], bf16, tag=f"c1in{h}", name=f"c1in{h}") for h in range(2)]
        c1o = apl.tile([128, NT, 10, 10], bf16, tag="c1o", name="c1o")
        c2o2 = apl.tile([128, NT, 18, 2, 10], bf16, tag="c2o2", name="c2o2")
        c3o = apl.tile([128, NT, 8, 34], bf16, tag="c3o", name="c3o")
        ot2 = [apl.tile([128, NT, 32], f32, tag=f"otile{q}", name=f"otile{q}")
               for q in range(2)]
        obuf = apl.tile([128, 8, 128], f32, tag="obuf", name="obuf")
        for h in range(2):
            nc.gpsimd.memset(c1in[h][:], 0.0)
        nc.gpsimd.memset(c1o[:].bitcast(f32), 0.0)
        nc.gpsimd.memset(c2o2[:].bitcast(f32), 0.0)
        nc.gpsimd.memset(c3o[:].bitcast(f32), 0.0)

        # ================= Wigner stage (one 128-sample batch) =================
        def wigner_a(s):
            a3 = a3t[s]
            # sx = sin(t/2 - pi/2), cx = sqrt(1 - sx^2)  for t = a, b, g
            sx = tp.tile([128, 3], f32, tag="sx", name="sx")
            cx = tp.tile([128, 3], f32, tag="cx", name="cx")
            sq = tp.tile([128, 3], f32, tag="sqt", name="sqt")
            nc.scalar.activation(sx[:], a3[:], AF.Sin, bias=c_nhpi[:], scale=c_half[:])
            nc.vector.tensor_mul(sq[:], sx[:], sx[:])
            nc.scalar.activation(cx[:], sq[:], AF.Sqrt, bias=c_one[:], scale=c_none[:])
            # half-angle of b: cb = -sx[:,1], sb = cx[:,1]
            cb = tp.tile([128, 1], f32, tag="cb", name="cb")
            sb = tp.tile([128, 1], f32, tag="sb", name="sb")
            nc.vector.tensor_scalar_mul(cb[:], sx[:, 1:2], -1.0)
            nc.vector.tensor_copy(sb[:], cx[:, 1:2])
            # full-angle cos/sin for a, g via double angle: cu = -sx, su = cx
            # c1 = 2 cu^2 - 1 ; s1 = 2 su cu
            cosT = tp.tile([128, 14], f32, tag=f"cosT{s}", name=f"cosT{s}")  # cols 0..6 cos(m a), 7..13 cos(m g)
            sinT = tp.tile([128, 14], f32, tag=f"sinT{s}", name=f"sinT{s}")
            nc.vector.memset(cosT[:, 0:1], 1.0)
            nc.vector.memset(cosT[:, 7:8], 1.0)
            nc.vector.memset(sinT[:, 0:1], 0.0)
            nc.vector.memset(sinT[:, 7:8], 0.0)
            cu = tp.tile([128, 2], f32, tag="cu", name="cu")
            su = tp.tile([128, 2], f32, tag="su", name="su")
            nc.vector.tensor_copy(cu[:], mk(sx, 0, [(2, 2)]))   # sx cols (a, g)
            nc.vector.tensor_copy(su[:], mk(cx, 0, [(2, 2)]))
            t0 = tp.tile([128, 2], f32, tag="t0", name="t0")
            nc.vector.tensor_mul(t0[:], cu[:], cu[:])
            c1 = tp.tile([128, 2], f32, tag="c1", name="c1")
            s1 = tp.tile([128, 2], f32, tag="s1", name="s1")
            nc.vector.tensor_scalar(c1[:], t0[:], 2.0, -1.0, op0=ALU.mult, op1=ALU.add)
            nc.vector.tensor_mul(t0[:], su[:], cu[:])
            nc.vector.tensor_scalar_mul(s1[:], t0[:], 2.0)
            # note cu = -sx is cos(t/2)? no: cu here = sx = -cos(t/2); but
            # c1 = 2 cu^2 - 1 = 2 cos^2(t/2) - 1 = cos(t) regardless of sign.
            # s1 = 2 su cu = 2 sin(t/2) (-cos(t/2)) = -sin(t) -> fix sign:
            nc.vector.tensor_scalar_mul(s1[:], s1[:], -1.0)
            cm = [None, (c1, 0)]
            # write m=1
            nc.vector.tensor_copy(mk(cosT, 1, [(7, 2)]), c1[:])
            nc.vector.tensor_copy(mk(sinT, 1, [(7, 2)]), s1[:])
            ta = tp.tile([128, 2], f32, tag="ta", name="ta")
            tb = tp.tile([128, 2], f32, tag="tb", name="tb")
            for m in range(2, 7):
                pcm = mk(cosT, m - 1, [(7, 2)])
                psm = mk(sinT, m - 1, [(7, 2)])
                nc.vector.tensor_mul(ta[:], pcm, c1[:])
                nc.vector.tensor_mul(tb[:], psm, s1[:])
                nc.vector.tensor_sub(mk(cosT, m, [(7, 2)]), ta[:], tb[:])
                nc.vector.tensor_mul(ta[:], psm, c1[:])
                nc.vector.tensor_mul(tb[:], pcm, s1[:])
                nc.vector.tensor_add(mk(sinT, m, [(7, 2)]), ta[:], tb[:])
            # power tables
            cpow = tp.tile([128, 13], f32, tag="cpow", name="cpow")
            spow = tp.tile([128, 13], f32, tag="spow", name="spow")
            for pw, base in ((cpow, cb), (spow, sb)):
                nc.vector.memset(pw[:, 0:1], 1.0)
                nc.vector.tensor_copy(pw[:, 1:2], base[:])
                xw = tp.tile([128, 1], f32, tag="xw", name="xw")
                nc.vector.tensor_mul(xw[:], base[:], base[:])
                nc.vector.tensor_scalar_mul(pw[:, 2:4], pw[:, 0:2], xw[:])
                nc.vector.tensor_mul(xw[:], xw[:], xw[:])
                nc.vector.tensor_scalar_mul(pw[:, 4:8], pw[:, 0:4], xw[:])
                nc.vector.tensor_mul(xw[:], xw[:], xw[:])
                nc.vector.tensor_scalar_mul(pw[:, 8:13], pw[:, 0:5], xw[:])
            # feat [128, 77] = [P49 | cosT 14 | sinT 14]
            feat = tp.tile([128, 103], f32, tag="feat", name="feat")
            for l in range(NL):
                n = 2 * l + 1
                nc.vector.tensor_mul(feat[:, OFF49[l]:OFF49[l] + n],
                                     mk(cpow, 2 * l, [(-1, n)]), spow[:, 0:n])
            nc.vector.tensor_copy(feat[:, 64:71], cosT[:, 7:14])
            nc.vector.tensor_copy(feat[:, 96:103], sinT[:, 7:14])
            # transpose -> base-0 lhsT tiles
            ptp = psum()
            nc.tensor.transpose(ptp[0:103, 0:128], feat[:, 0:103], ident[:])
            featP = tp.tile([49, 128], f32r, tag="featP", name="featP")
            featCG = tp.tile([7, 128], f32r, tag="featCG", name="featCG")
            featSG = tp.tile([7, 128], f32r, tag="featSG", name="featSG")
            nc.vector.tensor_copy(featP[:], ptp[0:49, 0:128])
            nc.vector.tensor_copy(featCG[:], ptp[64:71, 0:128])
            nc.vector.tensor_copy(featSG[:], ptp[96:103, 0:128])
            # dvec = P_all @ CC : [128, 455]
            pd = psum()
            nc.tensor.matmul(pd[:, 0:456], featP[:],
                             CC[:], start=True, stop=True)

            # y = T_g @ F : [128, 490]
            py = psum()
            nc.tensor.matmul(py[:, 0:490], featCG[:],
                             Fcs[:, 0:490], start=True, stop=False)
            nc.tensor.matmul(py[:, 0:490], featSG[:],
                             Fcs[:, 490:980], start=False, stop=True)
            # evacuate psums here so none are held across interleaved conv tiles
            y = tp.tile([128, 490], bf16, tag="y", name="y")
            nc.scalar.activation(y[:], py[:, 0:490], AF.Identity, bias=c_zero[:])
            dvec = tp.tile([128, 456], bf16, tag="dv", name="dvec")
            nc.scalar.activation(dvec[:], pd[:, 0:456], AF.Identity, bias=c_zero[:])
            return y, dvec, cosT, sinT

        def wigner_b(s, y, dvec, cosT, sinT):
            # t2[n,(u,r)] = sum_v d[n,(u,v)] y[n,(v,r)] via wide mul + X-axis reduce
            t2 = tp.tile([128, 490], f32, tag="t2", name="t2")
            prod = tp.tile([128, 1690], bf16, tag="prod", name="prod")
            nc.vector.tensor_mul(t2[:, 0:R], y[:, 0:R], mk(dvec, 0, [(0, R)]))
            for l in range(1, NL):
                n = 2 * l + 1
                d_ap = mk(dvec, OFF455[l], [(n, n), (0, R), (1, n)])
                y_ap = mk(y, OFF49[l] * R, [(0, n), (1, R), (R, n)])
                nc.vector.tensor_mul(mk(prod, 0, [(n * R, n), (n, R), (1, n)]),
                                     d_ap, y_ap)
                nc.vector.tensor_reduce(
                    mk(t2, OFF49[l] * R, [(R, n), (1, R)]),
                    mk(prod, 0, [(n * R, n), (n, R), (1, n)]),
                    axis=mybir.AxisListType.X, op=ALU.add)
            # Za apply: item = caE*t2 + saE*flip(t2)
            nsin = tp.tile([128, 7], f32, tag="nsin", name="nsin")
            nc.scalar.activation(nsin[:], sinT[:, 0:7], AF.Identity,
                                 bias=c_zero[:], scale=c_none[:])
            caE = tp.tile([128, 490], f32, tag="caE", name="caE")
            saE = tp.tile([128, 490], f32, tag="saE", name="saE")
            for l in range(NL):
                n = 2 * l + 1
                base = OFF49[l] * R
                nc.scalar.activation(caE[:, base:base + (l + 1) * R],
                                     mk(cosT, l, [(-1, l + 1), (0, R)]),
                                     AF.Identity, bias=c_zero[:])
                nc.scalar.activation(saE[:, base:base + (l + 1) * R],
                                     mk(nsin, l, [(-1, l + 1), (0, R)]),
                                     AF.Identity, bias=c_zero[:])
                if l > 0:
                    nc.scalar.activation(caE[:, base + l * R:base + n * R],
                                         mk(cosT, 0, [(1, l + 1), (0, R)]),
                                         AF.Identity, bias=c_zero[:])
                    nc.scalar.activation(saE[:, base + l * R:base + n * R],
                                         mk(sinT, 0, [(1, l + 1), (0, R)]),
                                         AF.Identity, bias=c_zero[:])
            item = tp.tile([128, 496], f32, tag="item", name="item")
            tmp2 = tp.tile([128, 490], f32, tag="tmpf", name="tmpf")
            nc.vector.memset(item[:, 490:491], 1.0)  # ones col -> dense bias row
            nc.vector.tensor_mul(item[:, 0:490], caE[:], t2[:])
            for l in range(NL):
                n = 2 * l + 1
                base = OFF49[l] * R
                nc.vector.tensor_mul(tmp2[:, base:base + n * R], saE[:, base:base + n * R],
                                     mk(t2, base + (n - 1) * R, [(-R, n), (1, R)]))
            nc.vector.tensor_add(item[:, 0:490], item[:, 0:490], tmp2[:])
            for kc in range(4):
                cnt = 128 if kc < 3 else 107
                pit = psum()
                nc.tensor.transpose(pit[0:cnt, 0:128], item[:, kc * 128:kc * 128 + cnt],
                                    ident[:])
                nc.scalar.activation(itemTb[kc][0:cnt, s * 128:(s + 1) * 128],
                                     pit[0:cnt, 0:128], AF.Identity,
                                     bias=c_zero[0:cnt, :])

        # ================= dense -> c1in [(c-half),(n,6,6)] bf16 =================
        def dense_half(s):
            for yy in range(4):
                for h in range(2):
                    pd2 = psum()
                    for xx in range(4):
                        mc = yy * 8 + xx * 2 + h
                        for kc in range(4):
                            nc.tensor.matmul(pd2[:, xx * 128:(xx + 1) * 128],
                                             Wk[kc][:, mc * 128:(mc + 1) * 128],
                                             itemTb[kc][:, s * 128:(s + 1) * 128],
                                             start=(kc == 0), stop=(kc == 3))
                    srcap = bass.AP(tensor=pd2[:].tensor, offset=pd2[:].offset,
                                    ap=[list(pd2[:].ap[0]), [1, 128], [128, 4]])
                    nc.scalar.activation(mk(c1in[h], (1 + yy) * 6 + 1 + s * 128 * 36,
                                            [(36, 128), (1, 4)]),
                                         srcap, AF.Relu, bias=c_zero[:])

        # ================= conv stack, one n-tile of 32 =================
        def conv_tile(t):
            ns = t * NT
            # conv1: psum [c128, (n32,4,4)]
            for pi4, (di, dj) in enumerate([(0, 0), (0, 1), (1, 0), (1, 1)]):
                ps = psum()
                tn = 0
                for si in _sis(di):
                    for sj in _sis(dj):
                        for h in range(2):
                            rhs = mk(c1in[h], ns * 36 + (1 + si) * 6 + (1 + sj),
                                     [(36, NT), (6, 4), (1, 4)])
                            nc.tensor.matmul(ps[:, 0:512], k1s[:, pi4, tn, h, :], rhs,
                                             start=(tn == 0 and h == 0),
                                             stop=(tn == 3 and h == 1))
                        tn += 1
                dst1 = mk(c1o, (1 + di) * 10 + (1 + dj),
                          [(100, NT), (20, 4), (2, 4)])
                if pi4 % 2 == 0:
                    nc.scalar.activation(dst1, ps[:, 0:512], AF.Relu, bias=b1[:])
                else:
                    nc.vector.tensor_scalar(dst1, ps[:, 0:512], b1[:], 0.0,
                                            op0=ALU.add, op1=ALU.max)
            # conv2 col-tiled: psum [(dj,ch,c32) via 4 col tiles, (n8,p8,pj8)]
            # out rows 2p+di (psum per di); tile j4=(dj,ch); acc (ri,t) exact taps
            for g in range(4):
                for di in range(2):
                    ps = psum()
                    for st, (ri, tt) in enumerate(((0, 0), (1, 0), (0, 1), (1, 1))):
                        for dj in range(2):
                            rhs = mk(c1o, (g * 8) * 100 + (di + ri) * 10 + dj + tt,
                                     [(100, 8), (10, 8), (1, 8)])
                            nc.tensor.matmul(ps[64 * dj:64 * dj + 64, 0:512],
                                             k2n[:, di, ri, tt, 64 * dj:64 * dj + 64],
                                             rhs, start=(st == 0), stop=(st == 3),
                                             tile_position=(0, 64 * dj))
                    # evac rsh0: rows r=2p+di at slot r+1; (dj,pj)-split cols, pj+1
                    for dj in range(2):
                        dst = mkp(c2o2, 0, 64,
                                  (g * 8) * 360 + (di + 1) * 20 + dj * 10 + 1,
                                  [(360, 8), (40, 8), (1, 8)])
                        src = ps[64 * dj:64 * dj + 64, 0:512]
                        bslc = b2[64 * dj:64 * dj + 64, :]
                        if dj == 0:
                            nc.scalar.activation(dst, src, AF.Relu, bias=bslc)
                        else:
                            nc.vector.tensor_scalar(dst, src, bslc, 0.0,
                                                    op0=ALU.add, op1=ALU.max)
                # rsh1 half = rows shifted one slot down, via sbuf->sbuf DMA
                nc.sync.dma_start(
                    out=mkp(c2o2, 64, 64, (g * 8) * 360, [(360, 8), (1, 320)]),
                    in_=mkp(c2o2, 0, 64, (g * 8) * 360 + 20, [(360, 8), (1, 320)]))
            # conv3 col-tiled: psum [(jm4,c32) via 4 col tiles, (n4,p16,qp8)]
            # K=(rsh,c64) covers row taps; acc t covers col taps exactly
            for di in range(2):
                for g in range(8):
                    ps = psum()
                    for tt in range(2):
                        for jm in range(4):
                            qloc, dj = jm >> 1, jm & 1
                            o = qloc + dj - 1 + tt
                            rhs = mk(c2o2, (g * 4) * 360 + di * 20
                                     + (o & 1) * 10 + (o >> 1) + 1,
                                     [(360, 4), (20, 16), (1, 8)])
                            nc.tensor.matmul(ps[32 * jm:32 * jm + 32, 0:512],
                                             k3n[:, di, tt, 32 * jm:32 * jm + 32],
                                             rhs, start=(tt == 0), stop=(tt == 1),
                                             tile_position=(0, 32 * jm))
                    dst3 = mk(c3o, (g * 4) * 272 + 1 + di,
                              [(272, 4), (34, 8), (2, 16)])
                    ps3 = bass.AP(tensor=ps[:].tensor, offset=ps[:].offset,
                                  ap=[list(ps[:].ap[0]), [128, 4], [1, 8], [8, 16]])
                    if g % 2 == 0:
                        nc.vector.tensor_scalar(dst3, ps3, b3[:],
                                                0.0, op0=ALU.add, op1=ALU.max)
                    else:
                        nc.scalar.activation(dst3, ps3, AF.Relu, bias=b3[:])
            # conv4 col-tiled: tile t4=(di,jh); psum m = di*64+jh*32+2*joutloc+dj
            # c2c outer so the first psum's bank frees while the second computes
            otile = ot2[t % 2]
            for c2c in range(2):
                ps4 = psum()
                for st in range(10):
                    bb, ri = st >> 1, st & 1
                    for t4 in range(4):
                        di, jh = t4 >> 1, t4 & 1
                        si = di - 1 + ri
                        rhs = mk(c3o, (c2c * 16) * 272 + (bb + 3 * jh) * 34 + 1 + si,
                                 [(272, 16), (1, 32)])
                        nc.tensor.matmul(ps4[32 * t4:32 * t4 + 32, 0:512],
                                         k4n[:, t4, ri, bb, :], rhs,
                                         start=(st == 0), stop=(st == 9),
                                         tile_position=(0, 32 * t4))
                # tail evac split across both engines to free the bank sooner
                dstA = mk(otile, (c2c * 16) * 32, [(32, 8), (1, 32)])
                dstB = mk(otile, (c2c * 16 + 8) * 32, [(32, 8), (1, 32)])
                nc.scalar.activation(dstA, ps4[:, 0:256], AF.Identity, bias=b4[:])
                nc.vector.tensor_scalar(dstB, ps4[:, 256:512], b4[:], 0.0,
                                        op0=ALU.add, op1=ALU.add)
        # output stage deferred: transposes queue after next tile's convs
        def conv_out(t):
            ns = t * NT
            otile = ot2[t % 2]
            for cq in range(2):
                pst = psum()
                for cc in range(4):
                    cch = cq * 4 + cc
                    nc.tensor.transpose(pst[0:128, cc * 128:cc * 128 + 128],
                                        otile[:, cch * 4:cch * 4 + 4, :].bitcast(f32),
                                        ident[:])
                nc.scalar.activation(obuf[:, cq * 4:cq * 4 + 4, :], pst[:, 0:512],
                                     AF.Identity, bias=c_zero[:])
            dst = bass.AP(tensor=out_d.tensor, offset=out_d.offset + ns * 4096,
                          ap=[[4096, 4], [128, 32], [16384, 8], [1, 128]])
            nc.sync.dma_start(out=dst, in_=obuf[:])

        # ================= orchestration: overlap wigner(s=1) with convs =================
        ya0 = wigner_a(0)
        wigner_b(0, *ya0)
        load_heavy()
        k1s, b1 = hw['k1s'], hw['b1']
        k2n, b2 = hw['k2n'], hw['b2']
        k3n, b3 = hw['k3n'], hw['b3']
        k4n, b4 = hw['k4n'], hw['b4']
        dense_half(0)
        conv_tile(0)
        ya1 = wigner_a(1)
        conv_tile(1)
        conv_out(0)
        wigner_b(1, *ya1)
        conv_tile(2)
        conv_out(1)
        dense_half(1)
        conv_tile(3)
        conv_out(2)
        for t in range(4, 8):
            conv_tile(t)
            conv_out(t - 1)
        conv_out(7)
        wdp.__exit__(None, None, None)
        ctx.close()
    nc.compile()
    return nc


_NC_CACHE = {}


def kernel(angles, item_rep, W, b, k1, b1, k2, b2, k3, b3, k4, b4):
    _install_axon_shim()
    from concourse.bass_utils import run_bass_kernel_spmd
    if 'nc' not in _NC_CACHE:
        _NC_CACHE['nc'] = _build()
    nc = _NC_CACHE['nc']
    wts = _prep_weights(np.asarray(item_rep, np.float32), np.asarray(W, np.float32),
                        np.asarray(b, np.float32), np.asarray(k1, np.float32),
                        np.asarray(b1, np.float32), np.asarray(k2, np.float32),
                        np.asarray(b2, np.float32), np.asarray(k3, np.float32),
                        np.asarray(b3, np.float32), np.asarray(k4, np.float32),
                        np.asarray(b4, np.float32))
    angles = np.asarray(angles, np.float32)
    in_maps = []
    for c in range(NCORES):
        m = dict(wts)
        m['angles'] = np.ascontiguousarray(angles[c * NPC:(c + 1) * NPC])
        in_maps.append(m)
    res = run_bass_kernel_spmd(nc, in_maps, core_ids=list(range(NCORES)))
    return np.concatenate([r['out'][:, None, :, :] for r in res.results], axis=0)



# revision 42
# speedup vs baseline: 1.3063x; 1.0150x over previous
"""Trainium2 Bass kernel for nn_ActionNet (Wigner-D block-diag rotation + dense +
4x stride-2 conv_transpose decoder), data-parallel over 8 NeuronCores.

Math: real Wigner D^l(a,b,g) = Zr(a) @ dr(b) @ Zr(g), with
  Zr(t): Zr[m,m]=cos(mt), Zr[l+m,l-m]=sin(mt), Zr[l-m,l+m]=-sin(mt)
  dr(b)[u,v] = sum_q Cr_l[u,v,q] cos(b/2)^(2l-q) sin(b/2)^q,  Cr_l = Re(B C_l B^H)
conv_transpose(s=2,k=4,SAME) phases (verified vs jax):
  out[2p+d]: d=0 -> K[2] x[p] + K[0] x[p-1];  d=1 -> K[1] x[p] + K[3] x[p+1]
Trig: sx = sin(t/2 - pi/2) (safe LUT domain), cx = sqrt(1-sx^2);
  cos(t/2) = -sx, sin(t/2) = cx; then double-angle + recurrence for cos/sin(m t).
"""
import math
import sys
import types

import numpy as np

sys.path.insert(0, '/opt/trn_rl_repo')
import ml_dtypes

DEGREES = 6
NL = DEGREES + 1
R = 10
N_BATCH = 2048
NCORES = 8
NPC = N_BATCH // NCORES
NT = 32
PI = math.pi
TAPS = {0: [(2, 0), (0, -1)], 1: [(1, 0), (3, 1)]}
OFF49 = [l * l for l in range(NL + 1)]
OFF455 = np.cumsum([0] + [(2 * l + 1) ** 2 for l in range(NL)]).tolist()


def _install_axon_shim():
    if 'antenv.axon_hooks' in sys.modules:
        return
    mod = types.ModuleType('antenv.axon_hooks')
    _h = [None]
    mod.set_axon_ntff_profile_hook = lambda h: _h.__setitem__(0, h)
    mod.get_axon_ntff_profile_hook = lambda: _h[0]
    sys.modules['antenv.axon_hooks'] = mod
    try:
        import antenv
        antenv.axon_hooks = mod
        from trn_agent_boot.trn_boot import _ntff_profile_via_ctypes
        mod.set_axon_ntff_profile_hook(_ntff_profile_via_ctypes('/opt/axon/libaxon_pjrt.so'))
    except Exception:
        pass


def _wigner_coeffs(l):
    f = math.factorial
    n = 2 * l + 1
    C = np.zeros((n, n, n))
    for mp in range(-l, l + 1):
        for m in range(-l, l + 1):
            pref = math.sqrt(f(l + mp) * f(l - mp) * f(l + m) * f(l - m))
            for s in range(max(0, m - mp), min(l + m, l - mp) + 1):
                q = mp - m + 2 * s
                den = f(l + m - s) * f(s) * f(mp - m + s) * f(l - mp - s)
                C[mp + l, m + l, q] += ((-1.0) ** (mp - m + s)) * pref / den
    return C


def _real_basis(l):
    n = 2 * l + 1
    B = np.zeros((n, n), dtype=np.complex128)
    B[l, l] = 1.0
    isq = 1.0 / math.sqrt(2.0)
    for m in range(1, l + 1):
        B[l + m, l + m] = ((-1) ** m) * isq
        B[l + m, l - m] = isq
        B[l - m, l - m] = 1j * isq
        B[l - m, l + m] = -1j * ((-1) ** m) * isq
    return B


def _build_consts():
    CC = np.zeros((49, 456), np.float32)
    M14 = np.zeros((7, 980), np.float32)
    for l in range(NL):
        C = _wigner_coeffs(l)
        B = _real_basis(l)
        Cr = np.real(np.einsum('ua,abq,vb->uvq', B, C, B.conj())).astype(np.float32)
        n = 2 * l + 1
        CC[OFF49[l]:OFF49[l] + n, OFF455[l]:OFF455[l] + n * n] = \
            np.moveaxis(Cr, 2, 0).reshape(n, n * n)
        for ul in range(n):
            m = ul - l
            v = OFF49[l] + ul
            M14[abs(m), v * R:(v + 1) * R] = 1.0
            if m != 0:
                M14[abs(m), 490 + v * R:490 + (v + 1) * R] = float(np.sign(m))
    return CC, M14


def _flip49(x):
    out = np.empty_like(x)
    for l in range(NL):
        out[OFF49[l]:OFF49[l + 1]] = x[OFF49[l]:OFF49[l + 1]][::-1]
    return out


def _ki(d, s):
    for k, ss in TAPS[d]:
        if ss == s:
            return k
    return None


def _sis(d):
    return [s for s in (-1, 0, 1) if _ki(d, s) is not None]


def _prep_weights(item_rep, W, b, k1, b1, k2, b2, k3, b3, k4, b4):
    bf16 = ml_dtypes.bfloat16
    inp = {}
    inp['repM'] = np.concatenate(
        [np.tile(item_rep.reshape(-1), (7, 1)),
         np.tile(_flip49(item_rep).reshape(-1), (7, 1))], 1).astype(np.float32)
    CC, M14 = _build_consts()
    inp['CC'], inp['M14'] = CC, M14
    inp['ident'] = np.eye(128, dtype=np.float32)
    Wp = np.zeros((512, 4096), np.float32)
    Wp[:490] = W
    Wp[490] = b  # bias row; itemT row 490 is set to ones
    inp['Wb'] = np.ascontiguousarray(Wp.reshape(4, 128, 4096)).astype(bf16)
    k1s = np.zeros((128, 4, 4, 2, 128), np.float32)
    for pi4, (di, dj) in enumerate([(0, 0), (0, 1), (1, 0), (1, 1)]):
        tn = 0
        for si in _sis(di):
            for sj in _sis(dj):
                kk = k1[_ki(di, si), _ki(dj, sj)]
                for h in range(2):
                    k1s[:, pi4, tn, h, :] = kk[h * 128:(h + 1) * 128]
                tn += 1
    inp['k1s'] = k1s.astype(bf16)
    inp['b1'] = b1.reshape(128, 1).astype(np.float32)
    # conv2 col-tiled: lhsT [cin128, di, ri, t, (dj,ch,co32)]
    k2n = np.zeros((128, 2, 2, 2, 128), np.float32)
    for di in range(2):
        for ri in range(2):
            for tt in range(2):
                for dj in range(2):
                    kk = k2[_ki(di, di - 1 + ri), _ki(dj, dj - 1 + tt)]
                    k2n[:, di, ri, tt, dj * 64:(dj + 1) * 64] = kk
    inp['k2n'] = k2n.astype(bf16)
    inp['b2'] = np.tile(b2, 2).reshape(128, 1).astype(np.float32)
    # conv3 col-tiled: lhsT [(rsh,cin64)128, di, t, (jm=(2*qloc+dj),co32)]
    k3n = np.zeros((2, 64, 2, 2, 4, 32), np.float32)
    for di in range(2):
        for rsh in range(2):
            for tt in range(2):
                for qloc in range(2):
                    for dj in range(2):
                        k3n[rsh, :, di, tt, 2 * qloc + dj, :] = \
                            k3[_ki(di, di - 1 + rsh), _ki(dj, dj - 1 + tt)]
    inp['k3n'] = k3n.reshape(128, 2, 2, 128).astype(bf16)
    inp['b3'] = np.tile(b3, 4).reshape(128, 1).astype(np.float32)
    # conv4 col-tiled banded: tile t4=(di,jh) covers m=di*64+jh*32+2*(jout-16jh)+dj
    # K=(jm,c32) block window; 5 blocks per tile, 2 exact row steps
    k4n = np.zeros((4, 32, 4, 2, 5, 32), np.float32)
    for t4 in range(4):
        di, jh = t4 >> 1, t4 & 1
        for ri in range(2):
            ki = _ki(di, di - 1 + ri)
            for bb in range(5):
                for jm in range(4):
                    j = 4 * (bb + 3 * jh) + jm
                    for mloc in range(32):
                        jout = 16 * jh + (mloc >> 1)
                        dj = mloc & 1
                        sj = j - jout
                        if sj in (dj - 1, dj):
                            k4n[jm, :, t4, ri, bb, mloc] = k4[ki, _ki(dj, sj), :, 0]
    inp['k4n'] = k4n.reshape(128, 4, 2, 5, 32).astype(bf16)
    inp['b4'] = np.full((128, 1), float(b4[0]), np.float32)
    return inp


def _build():
    import concourse.bass as bass
    import concourse.mybir as mybir
    import concourse.tile as tile
    from concourse import bacc
    import contextlib

    dt = mybir.dt
    AF = mybir.ActivationFunctionType
    ALU = mybir.AluOpType
    f32, f32r, bf16 = dt.float32, dt.float32r, dt.bfloat16
    nc = bacc.Bacc("TRN2", target_bir_lowering=False, debug=False, num_devices=NCORES)

    def din(name, shape, dtype=f32):
        return nc.dram_tensor(name, list(shape), dtype, kind="ExternalInput").ap()

    ang = din('angles', [NPC, 3])
    repM_d = din('repM', [7, 980])
    CC_d = din('CC', [49, 456], f32r)
    M14_d = din('M14', [7, 980])
    id_d = din('ident', [128, 128])
    Wb_d = din('Wb', [4, 128, 4096], bf16)
    k1s_d = din('k1s', [128, 4, 4, 2, 128], bf16)
    b1_d = din('b1', [128, 1])
    k2n_d = din('k2n', [128, 2, 2, 2, 128], bf16)
    b2_d = din('b2', [128, 1])
    k3n_d = din('k3n', [128, 2, 2, 128], bf16)
    b3_d = din('b3', [128, 1])
    k4n_d = din('k4n', [128, 4, 2, 5, 32], bf16)
    b4_d = din('b4', [128, 1])
    out_d = nc.dram_tensor('out', [NPC, 64, 64], f32, kind="ExternalOutput").ap()

    def mk(t, off, dims):
        a = t[:]
        return bass.AP(tensor=a.tensor, offset=a.offset + off,
                       ap=[[a.ap[0][0], a.ap[0][1]]] + [[s, c] for s, c in dims])

    def mkp(t, p0, pn, off, dims):
        a = t[:]
        return bass.AP(tensor=a.tensor, offset=a.offset + p0 * a.ap[0][0] + off,
                       ap=[[a.ap[0][0], pn]] + [[s, c] for s, c in dims])

    with tile.TileContext(nc) as tc:
        ctx = contextlib.ExitStack()
        wp = ctx.enter_context(tc.tile_pool(name="wts", bufs=1))
        apl = ctx.enter_context(tc.tile_pool(name="acts", bufs=1))
        tp = ctx.enter_context(tc.tile_pool(name="tmp", bufs=1))
        pp = ctx.enter_context(tc.tile_pool(name="ps", bufs=8, space="PSUM"))

        def psum():
            return pp.tile([128, 512], f32, tag="ps", name="ps")

        def load(dram_ap, shape, dtype=f32, tag=None):
            t = wp.tile(shape, dtype, tag=tag)
            nc.sync.dma_start(out=t[:], in_=dram_ap)
            return t

        a3t = []
        for s in range(2):
            at = wp.tile([128, 3], f32, tag=f"a3_{s}", name=f"a3_{s}")
            nc.sync.dma_start(out=at[:], in_=ang[s * 128:(s + 1) * 128, :])
            a3t.append(at)
        # y-stage weights, split so matmul lhsT/rhs share base partition 0
        wdp = tc.tile_pool(name="wdense", bufs=1)
        wdpo = wdp.__enter__()
        def loadw(dram_ap, shape, dtype=f32, tag=None):
            t = wdpo.tile(shape, dtype, tag=tag, name=tag)
            nc.sync.dma_start(out=t[:], in_=dram_ap)
            return t
        repM = loadw(repM_d[:, :], [7, 980], tag="repM")
        CC = loadw(CC_d[:, :], [49, 456], f32r, tag="CC")
        M14 = loadw(M14_d[:, :], [7, 980], tag="M14")
        Fcs = wdpo.tile([7, 980], f32r, tag="Fcs", name="Fcs")
        nc.vector.tensor_mul(Fcs[:], M14[:], repM[:])
        ident = load(id_d[:, :], [128, 128], tag="ident")
        identb = wp.tile([128, 128], bf16, tag="identb", name="identb")
        nc.vector.tensor_copy(identb[:], ident[:])
        hw = {}

        def load_heavy():
            # deferred so startup DMA doesn't gate the batch-0 wigner compute
            hw['k1s'] = load(k1s_d[:, :, :, :, :], [128, 4, 4, 2, 128], bf16, tag="k1s")
            hw['b1'] = load(b1_d[:, :], [128, 1], tag="b1")
            hw['k2n'] = load(k2n_d[:, :, :, :, :], [128, 2, 2, 2, 128], bf16, tag="k2n")
            hw['b2'] = load(b2_d[:, :], [128, 1], tag="b2")
            hw['k3n'] = load(k3n_d[:, :, :, :], [128, 2, 2, 128], bf16, tag="k3n")
            hw['b3'] = load(b3_d[:, :], [128, 1], tag="b3")
            hw['k4n'] = load(k4n_d[:, :, :, :, :], [128, 4, 2, 5, 32], bf16, tag="k4n")
            hw['b4'] = load(b4_d[:, :], [128, 1], tag="b4")

        c_half = wp.tile([128, 1], f32, tag="c_half", name="c_half")
        c_nhpi = wp.tile([128, 1], f32, tag="c_nhpi", name="c_nhpi")
        c_none = wp.tile([128, 1], f32, tag="c_none", name="c_none")
        c_one = wp.tile([128, 1], f32, tag="c_one", name="c_one")
        nc.vector.memset(c_half[:], 0.5)
        nc.vector.memset(c_nhpi[:], -PI / 2.0)
        nc.vector.memset(c_none[:], -1.0)
        nc.vector.memset(c_one[:], 1.0)
        c_zero = wp.tile([128, 1], f32, tag="c_zero", name="c_zero")
        nc.vector.memset(c_zero[:], 0.0)



        itemTb = [wdpo.tile([128, 256], bf16, tag=f"itemTb{kc}", name=f"itemTb{kc}") for kc in range(4)]
        nc.vector.memset(itemTb[3][:], 0.0)
        Wk = []
        for kc in range(4):
            wt = wdpo.tile([128, 4096], bf16, tag=f"Wk{kc}", name=f"Wk{kc}")
            nc.sync.dma_start(out=wt[:], in_=Wb_d[kc, :, :])
            Wk.append(wt)

        c1in = [apl.tile([128, NPC, 6, 6], bf16, tag=f"c1in{h}", name=f"c1in{h}") for h in range(2)]
        c1o = apl.tile([128, NT, 10, 10], bf16, tag="c1o", name="c1o")
        c2o2 = apl.tile([128, NT, 18, 2, 10], bf16, tag="c2o2", name="c2o2")
        c3o = apl.tile([128, NT, 8, 34], bf16, tag="c3o", name="c3o")
        ot2 = [apl.tile([128, NT, 32], bf16, tag=f"otile{q}", name=f"otile{q}")
               for q in range(2)]
        obuf = apl.tile([128, 8, 128], f32, tag="obuf", name="obuf")
        for h in range(2):
            nc.gpsimd.memset(c1in[h][:], 0.0)
        nc.gpsimd.memset(c1o[:].bitcast(f32), 0.0)
        nc.gpsimd.memset(c2o2[:].bitcast(f32), 0.0)
        nc.gpsimd.memset(c3o[:].bitcast(f32), 0.0)

        # ================= Wigner stage (one 128-sample batch) =================
        def wigner_a(s):
            a3 = a3t[s]
            # sx = sin(t/2 - pi/2), cx = sqrt(1 - sx^2)  for t = a, b, g
            sx = tp.tile([128, 3], f32, tag="sx", name="sx")
            cx = tp.tile([128, 3], f32, tag="cx", name="cx")
            sq = tp.tile([128, 3], f32, tag="sqt", name="sqt")
            nc.scalar.activation(sx[:], a3[:], AF.Sin, bias=c_nhpi[:], scale=c_half[:])
            nc.vector.tensor_mul(sq[:], sx[:], sx[:])
            nc.scalar.activation(cx[:], sq[:], AF.Sqrt, bias=c_one[:], scale=c_none[:])
            # half-angle of b: cb = -sx[:,1], sb = cx[:,1]
            cb = tp.tile([128, 1], f32, tag="cb", name="cb")
            sb = tp.tile([128, 1], f32, tag="sb", name="sb")
            nc.vector.tensor_scalar_mul(cb[:], sx[:, 1:2], -1.0)
            nc.vector.tensor_copy(sb[:], cx[:, 1:2])
            # full-angle cos/sin for a, g via double angle: cu = -sx, su = cx
            # c1 = 2 cu^2 - 1 ; s1 = 2 su cu
            cosT = tp.tile([128, 14], f32, tag=f"cosT{s}", name=f"cosT{s}")  # cols 0..6 cos(m a), 7..13 cos(m g)
            sinT = tp.tile([128, 14], f32, tag=f"sinT{s}", name=f"sinT{s}")
            nc.vector.memset(cosT[:, 0:1], 1.0)
            nc.vector.memset(cosT[:, 7:8], 1.0)
            nc.vector.memset(sinT[:, 0:1], 0.0)
            nc.vector.memset(sinT[:, 7:8], 0.0)
            cu = tp.tile([128, 2], f32, tag="cu", name="cu")
            su = tp.tile([128, 2], f32, tag="su", name="su")
            nc.vector.tensor_copy(cu[:], mk(sx, 0, [(2, 2)]))   # sx cols (a, g)
            nc.vector.tensor_copy(su[:], mk(cx, 0, [(2, 2)]))
            t0 = tp.tile([128, 2], f32, tag="t0", name="t0")
            nc.vector.tensor_mul(t0[:], cu[:], cu[:])
            c1 = tp.tile([128, 2], f32, tag="c1", name="c1")
            s1 = tp.tile([128, 2], f32, tag="s1", name="s1")
            nc.vector.tensor_scalar(c1[:], t0[:], 2.0, -1.0, op0=ALU.mult, op1=ALU.add)
            nc.vector.tensor_mul(t0[:], su[:], cu[:])
            nc.vector.tensor_scalar_mul(s1[:], t0[:], 2.0)
            # note cu = -sx is cos(t/2)? no: cu here = sx = -cos(t/2); but
            # c1 = 2 cu^2 - 1 = 2 cos^2(t/2) - 1 = cos(t) regardless of sign.
            # s1 = 2 su cu = 2 sin(t/2) (-cos(t/2)) = -sin(t) -> fix sign:
            nc.vector.tensor_scalar_mul(s1[:], s1[:], -1.0)
            # write m=1
            nc.vector.tensor_copy(mk(cosT, 1, [(7, 2)]), c1[:])
            nc.vector.tensor_copy(mk(sinT, 1, [(7, 2)]), s1[:])
            ta = tp.tile([128, 4], f32, tag="ta", name="ta")
            tb = tp.tile([128, 4], f32, tag="tb", name="tb")
            # m=2 via double angle
            nc.vector.tensor_mul(ta[:, 0:2], c1[:], c1[:])
            nc.vector.tensor_mul(tb[:, 0:2], s1[:], s1[:])
            nc.vector.tensor_sub(mk(cosT, 2, [(7, 2)]), ta[:, 0:2], tb[:, 0:2])
            nc.vector.tensor_mul(ta[:, 0:2], s1[:], c1[:])
            nc.vector.tensor_add(mk(sinT, 2, [(7, 2)]), ta[:, 0:2], ta[:, 0:2])
            # {m,m+1} = {m-2,m-1} + 2, vectorized over (pair, {a,g})
            for base in (1, 3):
                cin = mk(cosT, base, [(1, 2), (7, 2)])
                sn = mk(sinT, base, [(1, 2), (7, 2)])
                c2r = mk(cosT, 2, [(0, 2), (7, 2)])
                s2r = mk(sinT, 2, [(0, 2), (7, 2)])
                nc.vector.tensor_mul(ta[:], cin, c2r)
                nc.vector.tensor_mul(tb[:], sn, s2r)
                nc.vector.tensor_sub(mk(cosT, base + 2, [(1, 2), (7, 2)]), ta[:], tb[:])
                nc.vector.tensor_mul(ta[:], sn, c2r)
                nc.vector.tensor_mul(tb[:], cin, s2r)
                nc.vector.tensor_add(mk(sinT, base + 2, [(1, 2), (7, 2)]), ta[:], tb[:])
            # power tables
            cpow = tp.tile([128, 13], f32, tag="cpow", name="cpow")
            spow = tp.tile([128, 13], f32, tag="spow", name="spow")
            for pw, base in ((cpow, cb), (spow, sb)):
                nc.vector.memset(pw[:, 0:1], 1.0)
                nc.vector.tensor_copy(pw[:, 1:2], base[:])
                xw = tp.tile([128, 1], f32, tag="xw", name="xw")
                nc.vector.tensor_mul(xw[:], base[:], base[:])
                nc.vector.tensor_scalar_mul(pw[:, 2:4], pw[:, 0:2], xw[:])
                nc.vector.tensor_mul(xw[:], xw[:], xw[:])
                nc.vector.tensor_scalar_mul(pw[:, 4:8], pw[:, 0:4], xw[:])
                nc.vector.tensor_mul(xw[:], xw[:], xw[:])
                nc.vector.tensor_scalar_mul(pw[:, 8:13], pw[:, 0:5], xw[:])
            # feat [128, 77] = [P49 | cosT 14 | sinT 14]
            feat = tp.tile([128, 103], f32, tag="feat", name="feat")
            for l in range(NL):
                n = 2 * l + 1
                nc.vector.tensor_mul(feat[:, OFF49[l]:OFF49[l] + n],
                                     mk(cpow, 2 * l, [(-1, n)]), spow[:, 0:n])
            nc.vector.tensor_copy(feat[:, 64:71], cosT[:, 7:14])
            nc.vector.tensor_copy(feat[:, 96:103], sinT[:, 7:14])
            # transpose -> base-0 lhsT tiles
            ptp = psum()
            nc.tensor.transpose(ptp[0:103, 0:128], feat[:, 0:103], ident[:])
            featP = tp.tile([49, 128], f32r, tag="featP", name="featP")
            featCG = tp.tile([7, 128], f32r, tag="featCG", name="featCG")
            featSG = tp.tile([7, 128], f32r, tag="featSG", name="featSG")
            nc.vector.tensor_copy(featP[:], ptp[0:49, 0:128])
            nc.vector.tensor_copy(featCG[:], ptp[64:71, 0:128])
            nc.vector.tensor_copy(featSG[:], ptp[96:103, 0:128])
            # dvec = P_all @ CC : [128, 455]
            pd = psum()
            nc.tensor.matmul(pd[:, 0:456], featP[:],
                             CC[:], start=True, stop=True)

            # y = T_g @ F : [128, 490]
            py = psum()
            nc.tensor.matmul(py[:, 0:490], featCG[:],
                             Fcs[:, 0:490], start=True, stop=False)
            nc.tensor.matmul(py[:, 0:490], featSG[:],
                             Fcs[:, 490:980], start=False, stop=True)
            # evacuate psums here so none are held across interleaved conv tiles
            y = tp.tile([128, 490], bf16, tag="y", name="y")
            nc.scalar.activation(y[:], py[:, 0:490], AF.Identity, bias=c_zero[:])
            dvec = tp.tile([128, 456], bf16, tag="dv", name="dvec")
            nc.scalar.activation(dvec[:], pd[:, 0:456], AF.Identity, bias=c_zero[:])
            return y, dvec, cosT, sinT

        def wigner_b(s, y, dvec, cosT, sinT):
            # t2[n,(u,r)] = sum_v d[n,(u,v)] y[n,(v,r)] via wide mul + X-axis reduce
            t2 = tp.tile([128, 490], f32, tag="t2", name="t2")
            prod = tp.tile([128, 1690], bf16, tag="prod", name="prod")
            nc.vector.tensor_mul(t2[:, 0:R], y[:, 0:R], mk(dvec, 0, [(0, R)]))
            for l in range(1, NL):
                n = 2 * l + 1
                d_ap = mk(dvec, OFF455[l], [(n, n), (0, R), (1, n)])
                y_ap = mk(y, OFF49[l] * R, [(0, n), (1, R), (R, n)])
                nc.vector.tensor_mul(mk(prod, 0, [(n * R, n), (n, R), (1, n)]),
                                     d_ap, y_ap)
                nc.vector.tensor_reduce(
                    mk(t2, OFF49[l] * R, [(R, n), (1, R)]),
                    mk(prod, 0, [(n * R, n), (n, R), (1, n)]),
                    axis=mybir.AxisListType.X, op=ALU.add)
            # Za apply: item = caE*t2 + saE*flip(t2)
            nsin = tp.tile([128, 7], f32, tag="nsin", name="nsin")
            nc.scalar.activation(nsin[:], sinT[:, 0:7], AF.Identity,
                                 bias=c_zero[:], scale=c_none[:])
            caE = tp.tile([128, 490], f32, tag="caE", name="caE")
            saE = tp.tile([128, 490], f32, tag="saE", name="saE")
            for l in range(NL):
                n = 2 * l + 1
                base = OFF49[l] * R
                nc.scalar.activation(caE[:, base:base + (l + 1) * R],
                                     mk(cosT, l, [(-1, l + 1), (0, R)]),
                                     AF.Identity, bias=c_zero[:])
                nc.scalar.activation(saE[:, base:base + (l + 1) * R],
                                     mk(nsin, l, [(-1, l + 1), (0, R)]),
                                     AF.Identity, bias=c_zero[:])
                if l > 0:
                    nc.scalar.activation(caE[:, base + l * R:base + n * R],
                                         mk(cosT, 0, [(1, l + 1), (0, R)]),
                                         AF.Identity, bias=c_zero[:])
                    nc.scalar.activation(saE[:, base + l * R:base + n * R],
                                         mk(sinT, 0, [(1, l + 1), (0, R)]),
                                         AF.Identity, bias=c_zero[:])
            item = tp.tile([128, 496], f32, tag="item", name="item")
            tmp2 = tp.tile([128, 490], f32, tag="tmpf", name="tmpf")
            nc.vector.memset(item[:, 490:491], 1.0)  # ones col -> dense bias row
            nc.vector.tensor_mul(item[:, 0:490], caE[:], t2[:])
            for l in range(NL):
                n = 2 * l + 1
                base = OFF49[l] * R
                nc.vector.tensor_mul(tmp2[:, base:base + n * R], saE[:, base:base + n * R],
                                     mk(t2, base + (n - 1) * R, [(-R, n), (1, R)]))
            nc.vector.tensor_add(item[:, 0:490], item[:, 0:490], tmp2[:])
            for kc in range(4):
                cnt = 128 if kc < 3 else 107
                pit = psum()
                nc.tensor.transpose(pit[0:cnt, 0:128], item[:, kc * 128:kc * 128 + cnt],
                                    ident[:])
                nc.scalar.activation(itemTb[kc][0:cnt, s * 128:(s + 1) * 128],
                                     pit[0:cnt, 0:128], AF.Identity,
                                     bias=c_zero[0:cnt, :])

        # ================= dense -> c1in [(c-half),(n,6,6)] bf16 =================
        def dense_half(s):
            for yy in range(4):
                for h in range(2):
                    pd2 = psum()
                    for xx in range(4):
                        mc = yy * 8 + xx * 2 + h
                        for kc in range(4):
                            nc.tensor.matmul(pd2[:, xx * 128:(xx + 1) * 128],
                                             Wk[kc][:, mc * 128:(mc + 1) * 128],
                                             itemTb[kc][:, s * 128:(s + 1) * 128],
                                             start=(kc == 0), stop=(kc == 3))
                    srcap = bass.AP(tensor=pd2[:].tensor, offset=pd2[:].offset,
                                    ap=[list(pd2[:].ap[0]), [1, 128], [128, 4]])
                    nc.scalar.activation(mk(c1in[h], (1 + yy) * 6 + 1 + s * 128 * 36,
                                            [(36, 128), (1, 4)]),
                                         srcap, AF.Relu, bias=c_zero[:])

        # ================= conv stack, one n-tile of 32 =================
        def conv_tile(t):
            ns = t * NT
            # conv1: psum [c128, (n32,4,4)]
            for pi4, (di, dj) in enumerate([(0, 0), (0, 1), (1, 0), (1, 1)]):
                ps = psum()
                tn = 0
                for si in _sis(di):
                    for sj in _sis(dj):
                        for h in range(2):
                            rhs = mk(c1in[h], ns * 36 + (1 + si) * 6 + (1 + sj),
                                     [(36, NT), (6, 4), (1, 4)])
                            nc.tensor.matmul(ps[:, 0:512], k1s[:, pi4, tn, h, :], rhs,
                                             start=(tn == 0 and h == 0),
                                             stop=(tn == 3 and h == 1))
                        tn += 1
                dst1 = mk(c1o, (1 + di) * 10 + (1 + dj),
                          [(100, NT), (20, 4), (2, 4)])
                if pi4 % 2 == 0:
                    nc.scalar.activation(dst1, ps[:, 0:512], AF.Relu, bias=b1[:])
                else:
                    nc.vector.tensor_scalar(dst1, ps[:, 0:512], b1[:], 0.0,
                                            op0=ALU.add, op1=ALU.max)
            # conv2 col-tiled: psum [(dj,ch,c32) via 4 col tiles, (n8,p8,pj8)]
            # out rows 2p+di (psum per di); tile j4=(dj,ch); acc (ri,t) exact taps
            for g in range(4):
                for di in range(2):
                    ps = psum()
                    for st, (ri, tt) in enumerate(((0, 0), (1, 0), (0, 1), (1, 1))):
                        for dj in range(2):
                            rhs = mk(c1o, (g * 8) * 100 + (di + ri) * 10 + dj + tt,
                                     [(100, 8), (10, 8), (1, 8)])
                            nc.tensor.matmul(ps[64 * dj:64 * dj + 64, 0:512],
                                             k2n[:, di, ri, tt, 64 * dj:64 * dj + 64],
                                             rhs, start=(st == 0), stop=(st == 3),
                                             tile_position=(0, 64 * dj))
                    # evac rsh0: rows r=2p+di at slot r+1; (dj,pj)-split cols, pj+1
                    for dj in range(2):
                        dst = mkp(c2o2, 0, 64,
                                  (g * 8) * 360 + (di + 1) * 20 + dj * 10 + 1,
                                  [(360, 8), (40, 8), (1, 8)])
                        src = ps[64 * dj:64 * dj + 64, 0:512]
                        bslc = b2[64 * dj:64 * dj + 64, :]
                        if dj == 0:
                            nc.scalar.activation(dst, src, AF.Relu, bias=bslc)
                        else:
                            nc.vector.tensor_scalar(dst, src, bslc, 0.0,
                                                    op0=ALU.add, op1=ALU.max)
                # rsh1 half = rows shifted one slot down, via sbuf->sbuf DMA
                nc.sync.dma_start(
                    out=mkp(c2o2, 64, 64, (g * 8) * 360, [(360, 8), (1, 320)]),
                    in_=mkp(c2o2, 0, 64, (g * 8) * 360 + 20, [(360, 8), (1, 320)]))
            # conv3 col-tiled: psum [(jm4,c32) via 4 col tiles, (n4,p16,qp8)]
            # K=(rsh,c64) covers row taps; acc t covers col taps exactly
            for di in range(2):
                for g in range(8):
                    ps = psum()
                    for tt in range(2):
                        for jm in range(4):
                            qloc, dj = jm >> 1, jm & 1
                            o = qloc + dj - 1 + tt
                            rhs = mk(c2o2, (g * 4) * 360 + di * 20
                                     + (o & 1) * 10 + (o >> 1) + 1,
                                     [(360, 4), (20, 16), (1, 8)])
                            nc.tensor.matmul(ps[32 * jm:32 * jm + 32, 0:512],
                                             k3n[:, di, tt, 32 * jm:32 * jm + 32],
                                             rhs, start=(tt == 0), stop=(tt == 1),
                                             tile_position=(0, 32 * jm))
                    dst3 = mk(c3o, (g * 4) * 272 + 1 + di,
                              [(272, 4), (34, 8), (2, 16)])
                    ps3 = bass.AP(tensor=ps[:].tensor, offset=ps[:].offset,
                                  ap=[list(ps[:].ap[0]), [128, 4], [1, 8], [8, 16]])
                    if g % 2 == 0:
                        nc.vector.tensor_scalar(dst3, ps3, b3[:],
                                                0.0, op0=ALU.add, op1=ALU.max)
                    else:
                        nc.scalar.activation(dst3, ps3, AF.Relu, bias=b3[:])
            # conv4 col-tiled: tile t4=(di,jh); psum m = di*64+jh*32+2*joutloc+dj
            # c2c outer so the first psum's bank frees while the second computes
            otile = ot2[t % 2]
            for c2c in range(2):
                ps4 = psum()
                for st in range(10):
                    bb, ri = st >> 1, st & 1
                    for t4 in range(4):
                        di, jh = t4 >> 1, t4 & 1
                        si = di - 1 + ri
                        rhs = mk(c3o, (c2c * 16) * 272 + (bb + 3 * jh) * 34 + 1 + si,
                                 [(272, 16), (1, 32)])
                        nc.tensor.matmul(ps4[32 * t4:32 * t4 + 32, 0:512],
                                         k4n[:, t4, ri, bb, :], rhs,
                                         start=(st == 0), stop=(st == 9),
                                         tile_position=(0, 32 * t4))
                # tail evac split across both engines to free the bank sooner
                dstA = mk(otile, (c2c * 16) * 32, [(32, 8), (1, 32)])
                dstB = mk(otile, (c2c * 16 + 8) * 32, [(32, 8), (1, 32)])
                nc.scalar.activation(dstA, ps4[:, 0:256], AF.Identity, bias=b4[:])
                nc.vector.tensor_scalar(dstB, ps4[:, 256:512], b4[:], 0.0,
                                        op0=ALU.add, op1=ALU.add)
        # output stage deferred: transposes queue after next tile's convs
        def conv_out(t):
            ns = t * NT
            otile = ot2[t % 2]
            for cq in range(2):
                pst = psum()
                bc = pst[:].bitcast(bf16)
                for cc in range(4):
                    cch = cq * 4 + cc
                    dstT = bass.AP(tensor=bc.tensor, offset=bc.offset + cc * 128,
                                   ap=[[bc.ap[0][0], 128], [1, 128]])
                    nc.tensor.transpose(dstT, otile[:, cch * 4:cch * 4 + 4, :],
                                        identb[:])
                nc.scalar.activation(obuf[:, cq * 4:cq * 4 + 4, :],
                                     bass.AP(tensor=bc.tensor, offset=bc.offset,
                                             ap=[[bc.ap[0][0], 128], [1, 512]]),
                                     AF.Identity, bias=c_zero[:])
            dst = bass.AP(tensor=out_d.tensor, offset=out_d.offset + ns * 4096,
                          ap=[[4096, 4], [128, 32], [16384, 8], [1, 128]])
            nc.sync.dma_start(out=dst, in_=obuf[:])

        # ================= orchestration: overlap wigner(s=1) with convs =================
        ya0 = wigner_a(0)
        wigner_b(0, *ya0)
        load_heavy()
        k1s, b1 = hw['k1s'], hw['b1']
        k2n, b2 = hw['k2n'], hw['b2']
        k3n, b3 = hw['k3n'], hw['b3']
        k4n, b4 = hw['k4n'], hw['b4']
        dense_half(0)
        conv_tile(0)
        ya1 = wigner_a(1)
        conv_tile(1)
        conv_out(0)
        wigner_b(1, *ya1)
        conv_tile(2)
        conv_out(1)
        dense_half(1)
        conv_tile(3)
        conv_out(2)
        for t in range(4, 8):
            conv_tile(t)
            conv_out(t - 1)
        conv_out(7)
        wdp.__exit__(None, None, None)
        ctx.close()
    nc.compile()
    return nc


_NC_CACHE = {}


def kernel(angles, item_rep, W, b, k1, b1, k2, b2, k3, b3, k4, b4):
    _install_axon_shim()
    from concourse.bass_utils import run_bass_kernel_spmd
    if 'nc' not in _NC_CACHE:
        _NC_CACHE['nc'] = _build()
    nc = _NC_CACHE['nc']
    wts = _prep_weights(np.asarray(item_rep, np.float32), np.asarray(W, np.float32),
                        np.asarray(b, np.float32), np.asarray(k1, np.float32),
                        np.asarray(b1, np.float32), np.asarray(k2, np.float32),
                        np.asarray(b2, np.float32), np.asarray(k3, np.float32),
                        np.asarray(b3, np.float32), np.asarray(k4, np.float32),
                        np.asarray(b4, np.float32))
    angles = np.asarray(angles, np.float32)
    in_maps = []
    for c in range(NCORES):
        m = dict(wts)
        m['angles'] = np.ascontiguousarray(angles[c * NPC:(c + 1) * NPC])
        in_maps.append(m)
    res = run_bass_kernel_spmd(nc, in_maps, core_ids=list(range(NCORES)))
    return np.concatenate([r['out'][:, None, :, :] for r in res.results], axis=0)

